# revision 14
# baseline (speedup 1.0000x reference)
"""Trainium2 Bass kernel for the CapacityNN PINN forward pass (v2).

Computes, for N = B*S collocation points x = (s, t):
  U   = MLP([s_norm, t_norm]) * tgt_std + tgt_mean
  F   = U_t  - G(U)             (G = Verhulst logistic growth term)
  F_t = U_tt - G'(U) * U_t
with U_t/U_tt computed exactly by forward-mode 2nd-order jet propagation
through the tanh MLP.

Sharding: pure data parallel over 8 NeuronCores (8192 points/core),
MLP weights + PDE scalars replicated (host-folded).

v2 layout/engine plan (from TimelineSim cost-model analysis):
  - all streams fp16 (DVE 2x tensor_tensor / 4x tensor_scalar modes)
  - [128, 1024]-wide PSUM tiles -> one Act/DVE op per stream per layer
  - elementwise jet algebra balanced across Act / DVE / Pool:
      Act : tanh, st=2*z1^2 (PSUM reads), 2 of 8 ee squares, final copies
      DVE : ee, dm=1-ee, tt=av*st, h2=dm*qt, ad0, 4 of 6 h1=dm*z1
      Pool: qt=z2-tt (PSUM read), 2 of 6 h1
  - all scalar prep + weight folding done on HOST; weights arrive as two
    pre-packed SBUF-image blocks (2 big DMAs instead of ~60 small ones)
  - tail transpose ([3,NLOC] -> [128,3*PPP]) streamed per-chunk via
    SBUF->SBUF DMAs overlapped with compute
"""

import os
import sys
import tempfile

import numpy as np

for _p in ("/opt/trn_rl_repo", "/root/.axon_site/_ro/trn_rl_repo"):
    if os.path.isdir(_p) and _p not in sys.path:
        sys.path.insert(0, _p)

import concourse.bass as bass
import concourse.bacc as bacc
import concourse.tile as tile
from concourse import mybir
from concourse.bass_utils import run_bass_kernel_spmd

AF = mybir.ActivationFunctionType
OP = mybir.AluOpType
F32 = mybir.dt.float32
F16 = mybir.dt.float16

NCORES = 8
B, S, H = 512, 128, 256
N = B * S                  # 65536 points
NLOC = N // NCORES         # 8192 points per core
CH = 1024                  # points per on-chip chunk
NCHUNK = NLOC // CH
PPP = NLOC // 128          # points per partition in the tail layout (64)
CPC = CH // 128            # tail cols per chunk (8)
SQRT2 = float(np.sqrt(2.0))

# ---- packed fp16 const block column map (must match _pack_w16) ----
# 21 [128,128] weight tiles (incl negid), then w0ts (rows 0-1), then w4 (2 cols)
_W16_TILES = []  # (name, col) in order
_c = 0
for _l in (1, 2, 3):
    for _kk in range(2):
        for _mm in range(2):
            _W16_TILES.append((f"wt{_l}_{_kk}{_mm}", _c))
            _c += 128
for _nm in ("wtw", "wtw2"):
    for _kk in range(2):
        for _mm in range(2):
            _W16_TILES.append((f"{_nm}_{_kk}{_mm}", _c))
            _c += 128
_W16_TILES.append(("negid", _c))
_c += 128
_W16_W0TS = _c          # [2, 256] at rows 0-1, cols [_c, _c+256)
_c += 256
_W16_W4 = _c            # [128, 2]: col kk = W4 half kk
_c += 2
W16COLS = _c

# ---- packed fp32 const block column map (must match _pack_w32) ----
# [128, W32COLS]: per-partition scalars and biases, one col each
_W32_NAMES = [
    "beta0_0", "beta0_1",
    "bl1_0", "bl1_1", "bl2_0", "bl2_1", "bl3_0", "bl3_1",
    "C_t", "c1", "nr", "mc3", "sts", "tmb",
]
W32COLS = len(_W32_NAMES)
_W32_IDX = {n: i for i, n in enumerate(_W32_NAMES)}


def _build():
    nc = bacc.Bacc(
        "TRN2",
        target_bir_lowering=False,
        debug=False,
        enable_asserts=False,
        num_devices=NCORES,
    )

    x2 = nc.dram_tensor("x2", [2, NLOC], F16, kind="ExternalInput").ap()
    wblk16 = nc.dram_tensor("wblk16", [128, W16COLS], F16, kind="ExternalInput").ap()
    wblk32 = nc.dram_tensor("wblk32", [128, W32COLS], F32, kind="ExternalInput").ap()
    out = nc.dram_tensor("out", [3, NLOC], F32, kind="ExternalOutput").ap()

    with tile.TileContext(nc) as tc:
        from contextlib import ExitStack

        with ExitStack() as ctx:
            const = ctx.enter_context(tc.tile_pool(name="const", bufs=1))
            sb = ctx.enter_context(tc.tile_pool(name="sb", bufs=1))
            ps = ctx.enter_context(tc.tile_pool(name="ps", bufs=1, space="PSUM"))

            # ---------- const loads: two big DMAs ----------
            w16 = const.tile([128, W16COLS], F16, name="w16")
            w32 = const.tile([128, W32COLS], F32, name="w32")
            nc.sync.dma_start(out=w16, in_=wblk16)
            nc.sync.dma_start(out=w32, in_=wblk32)

            wtile = {}
            for nm, col in _W16_TILES:
                wtile[nm] = w16[:, col : col + 128]
            w0ts = w16[0:2, _W16_W0TS : _W16_W0TS + 256]  # [2, 256]
            w4c = [w16[:, _W16_W4 + kk : _W16_W4 + kk + 1] for kk in range(2)]

            def sc(name):
                i = _W32_IDX[name]
                return w32[:, i : i + 1]

            beta0 = [sc("beta0_0"), sc("beta0_1")]
            bl = {l: [sc(f"bl{l}_0"), sc(f"bl{l}_1")] for l in (1, 2, 3)}
            C_t, c1, nr, mc3, sts, tmb = (
                sc("C_t"), sc("c1"), sc("nr"), sc("mc3"), sc("sts"), sc("tmb"),
            )

            # ---------- main loop over point chunks ----------
            # oc[p, c*24 + b*3 + s] = stream-s output for point c*CH + b*128 + p
            oc = sb.tile([128, 3 * PPP], F32, name="oc")

            for c in range(NCHUNK):
                x2c = sb.tile([2, CH], F16, tag="x2c", bufs=2)
                nc.sync.dma_start(out=x2c, in_=x2[:, c * CH : (c + 1) * CH])

                # ----- layer 0: primal only; tangents fold into layer-1
                # weights (H1 <- dm, H2 <- ad) -----
                Hv = [None] * 2
                H1 = [None] * 2
                H2 = [None] * 2
                for m in range(2):
                    pz = ps.tile([128, CH], F32, tag="pz", bufs=3, name="pz0")
                    for g in range(2):
                        nc.tensor.matmul(
                            pz[:, g * 512 : (g + 1) * 512],
                            w0ts[:, m * 128 : (m + 1) * 128],
                            x2c[:, g * 512 : (g + 1) * 512],
                            start=True,
                            stop=True,
                        )
                    av = sb.tile([128, CH], F16, tag=f"hv{m}", bufs=3, name="av")
                    ee = sb.tile([128, CH], F16, tag=f"ee{m}", bufs=2, name="ee")
                    dm = sb.tile([128, CH], F16, tag=f"dm{m}", bufs=2, name="dm")
                    ad = sb.tile([128, CH], F16, tag=f"ad{m}", bufs=2, name="ad")
                    nc.scalar.activation(av, pz, AF.Tanh, beta0[m])
                    nc.vector.tensor_tensor(ee, av, av, OP.mult)
                    nc.vector.tensor_scalar(dm, ee, -1.0, 1.0, OP.mult, OP.add)
                    nc.vector.tensor_tensor(ad, av, dm, OP.mult)
                    Hv[m], H1[m], H2[m] = av, dm, ad

                # ----- hidden layers 1..3 -----
                for l in (1, 2, 3):
                    nHv = [None] * 2
                    nDm = [None] * 2
                    nH1 = [None] * 2
                    nH2 = [None] * 2
                    Tt = [None] * 2
                    # primal
                    for m in range(2):
                        pz = ps.tile([128, CH], F32, tag="pz", bufs=3, name="pzv")
                        for g in range(2):
                            for kk in range(2):
                                nc.tensor.matmul(
                                    pz[:, g * 512 : (g + 1) * 512],
                                    wtile[f"wt{l}_{kk}{m}"],
                                    Hv[kk][:, g * 512 : (g + 1) * 512],
                                    start=(kk == 0),
                                    stop=(kk == 1),
                                )
                        av = sb.tile([128, CH], F16, tag=f"hv{m}", bufs=3, name="av")
                        ee = sb.tile([128, CH], F16, tag=f"ee{m}", bufs=2, name="ee")
                        dm = sb.tile([128, CH], F16, tag=f"dm{m}", bufs=2, name="dm")
                        nc.scalar.activation(av, pz, AF.Tanh, bl[l][m])
                        if l == 1 or (l == 2 and m == 0):
                            # rebalance: 3 of 8 squares on Act
                            nc.scalar.activation(ee, av, AF.Square)
                        else:
                            nc.vector.tensor_tensor(ee, av, av, OP.mult)
                        nc.vector.tensor_scalar(dm, ee, -1.0, 1.0, OP.mult, OP.add)
                        nHv[m], nDm[m] = av, dm
                    # first-derivative stream
                    for m in range(2):
                        pz1 = ps.tile([128, CH], F32, tag="pz", bufs=3, name="pz1")
                        w1nm = f"wtw_{{}}{m}" if l == 1 else f"wt{l}_{{}}{m}"
                        for g in range(2):
                            for kk in range(2):
                                nc.tensor.matmul(
                                    pz1[:, g * 512 : (g + 1) * 512],
                                    wtile[w1nm.format(kk)],
                                    H1[kk][:, g * 512 : (g + 1) * 512],
                                    start=(kk == 0),
                                    stop=(kk == 1),
                                )
                        st = sb.tile([128, CH], F16, tag=f"st{m}", bufs=2, name="st")
                        h1t = sb.tile([128, CH], F16, tag=f"h1{m}", bufs=3, name="h1t")
                        tt = sb.tile([128, CH], F16, tag=f"tt{m}", bufs=2, name="tt")
                        nc.scalar.activation(st, pz1, AF.Square, 0.0, SQRT2)  # 2*z1^2
                        nc.vector.tensor_tensor(h1t, nDm[m], pz1, OP.mult)
                        nc.gpsimd.tensor_tensor(tt, nHv[m], st, OP.mult)  # a*st
                        nH1[m], Tt[m] = h1t, tt
                    # second-derivative stream; qt = z2 - tt computed on PE by
                    # accumulating -I @ tt into the z2 psum group
                    for m in range(2):
                        pz2 = ps.tile([128, CH], F32, tag="pz", bufs=3, name="pz2")
                        w2nm = f"wtw2_{{}}{m}" if l == 1 else f"wt{l}_{{}}{m}"
                        for g in range(2):
                            for kk in range(2):
                                nc.tensor.matmul(
                                    pz2[:, g * 512 : (g + 1) * 512],
                                    wtile[w2nm.format(kk)],
                                    H2[kk][:, g * 512 : (g + 1) * 512],
                                    start=(kk == 0),
                                    stop=False,
                                )
                            nc.tensor.matmul(
                                pz2[:, g * 512 : (g + 1) * 512],
                                wtile["negid"],
                                Tt[m][:, g * 512 : (g + 1) * 512],
                                start=False,
                                stop=True,
                            )
                        h2t = sb.tile([128, CH], F16, tag=f"h2{m}", bufs=3, name="h2t")
                        nc.vector.tensor_tensor(h2t, nDm[m], pz2, OP.mult)  # d*(z2-tt)
                        nH2[m] = h2t
                    Hv, H1, H2 = nHv, nH1, nH2

                # ----- final projection, transposed: per 128-point block b,
                # out col s of pblk = stream_s . w4  (1-row matmuls) -----
                pblk = ps.tile([128, 3 * CPC], F32, tag="pb", bufs=2, name="pblk")
                for b in range(CPC):
                    for s_idx, stream in enumerate((Hv, H1, H2)):
                        for kk in range(2):
                            nc.tensor.matmul(
                                pblk[:, b * 3 + s_idx : b * 3 + s_idx + 1],
                                stream[kk][:, b * 128 : (b + 1) * 128],
                                w4c[kk],
                                start=(kk == 0),
                                stop=(kk == 1),
                            )
                nc.scalar.copy(oc[:, c * 3 * CPC : (c + 1) * 3 * CPC], pblk)

            # ----- tail (once): PDE algebra on [128, 3*PPP], strided views -----
            oc2 = sb.tile([128, 3 * PPP], F32, name="oc2")

            yv, yt, ytt = (oc[:, s : 3 * PPP : 3] for s in range(3))
            U, Fo, Ft = (oc2[:, s : 3 * PPP : 3] for s in range(3))

            def tl(name):
                return sb.tile([128, PPP], F32, name=name)

            ut, utt, vv, v2, w1, q1, t1 = (
                tl("ut"), tl("utt"), tl("vv"), tl("v2"), tl("w1"), tl("q1"), tl("t1"),
            )
            nc.vector.tensor_scalar(U, yv, sts, tmb, OP.mult, OP.add)
            nc.vector.tensor_scalar(ut, yt, sts, None, OP.mult)
            nc.vector.tensor_scalar(utt, ytt, sts, None, OP.mult)
            nc.vector.tensor_scalar(vv, U, C_t, None, OP.subtract)
            nc.vector.tensor_tensor(v2, vv, vv, OP.mult)
            nc.vector.scalar_tensor_tensor(w1, v2, c1, vv, OP.mult, OP.add)
            nc.vector.scalar_tensor_tensor(Fo, w1, nr, ut, OP.mult, OP.add)
            nc.vector.tensor_tensor(q1, vv, ut, OP.mult)
            nc.vector.scalar_tensor_tensor(t1, ut, nr, utt, OP.mult, OP.add)
            nc.vector.scalar_tensor_tensor(Ft, q1, mc3, t1, OP.mult, OP.add)

            # out[s, c*CH + b*128 + p] = oc2[p, c*3*CPC + b*3 + s]
            for s_idx in range(3):
                nc.sync.dma_start(
                    out=bass.AP(
                        out.tensor,
                        s_idx * NLOC,
                        [[1, 128], [CH, NCHUNK], [128, CPC]],
                    ),
                    in_=bass.AP(
                        oc2.tensor,
                        oc2.offset + s_idx,
                        [list(oc2.ap[0]), [3 * CPC, NCHUNK], [3, CPC]],
                    ),
                )

    nc.compile()
    return nc


_STATE = {}


def _get_nc():
    if "nc" not in _STATE:
        _STATE["nc"] = _build()
    return _STATE["nc"]


def _pack_consts(inputs):
    f = np.float32

    def arr(k):
        return np.ascontiguousarray(np.asarray(inputs[k], f))

    W0, b0 = arr("W0"), arr("b0")
    Ws = {1: arr("W1"), 2: arr("W2"), 3: arr("W3")}
    bs = {1: arr("b1"), 2: arr("b2"), 3: arr("b3")}
    W4, b4 = arr("W4").reshape(1, H), arr("b4").reshape(1)
    in_mean, in_std = arr("in_mean"), arr("in_std")
    tgt_mean, tgt_std = arr("tgt_mean"), arr("tgt_std")
    lgr = float(arr("log_growth_rate").reshape(-1)[0])
    lcc = float(arr("log_carrying_capacity").reshape(-1)[0])
    lil = float(arr("log_initial_loss").reshape(-1)[0])

    # fp16 block
    w16 = np.zeros((128, W16COLS), np.float16)
    for nm, col in _W16_TILES:
        if nm == "negid":
            tilev = -np.eye(128, dtype=np.float32)
        elif nm.startswith("wtw"):
            base, km = nm.rsplit("_", 1)
            kk, mm = int(km[0]), int(km[1])
            if base == "wtw":
                Wf = (Ws[1] * W0[:, 1][None, :]).T
            else:
                Wf = (Ws[1] * (-2.0 * W0[:, 1] ** 2)[None, :]).T
            tilev = Wf[kk * 128 : (kk + 1) * 128, mm * 128 : (mm + 1) * 128]
        else:
            l, km = nm[2:].split("_")
            l, kk, mm = int(l), int(km[0]), int(km[1])
            Wt = Ws[l].T  # [in, out]
            tilev = Wt[kk * 128 : (kk + 1) * 128, mm * 128 : (mm + 1) * 128]
        w16[:, col : col + 128] = tilev.astype(np.float16)
    # w0ts: W0.T rows scaled by 1/(std+eps)
    w0ts = (W0.T / (in_std[:, None] + 1e-8)).astype(np.float16)  # [2, H]
    w16[0:2, _W16_W0TS : _W16_W0TS + 256] = w0ts
    # w4 halves as [128, 2]; negid handled in the tile loop below
    for kk in range(2):
        w16[:, _W16_W4 + kk] = W4[0, kk * 128 : (kk + 1) * 128].astype(np.float16)

    # fp32 block
    w32 = np.zeros((128, W32COLS), np.float32)

    def put(name, vec):
        w32[:, _W32_IDX[name]] = vec

    m0i = in_mean[0] / (in_std[0] + 1e-8)
    m1i = in_mean[1] / (in_std[1] + 1e-8)
    u = W0[:, 0] * m0i + W0[:, 1] * m1i
    beta0 = b0 - u
    put("beta0_0", beta0[0:128])
    put("beta0_1", beta0[128:256])
    for l in (1, 2, 3):
        put(f"bl{l}_0", bs[l][0:128])
        put(f"bl{l}_1", bs[l][128:256])
    r = np.exp(-lgr)
    K = 0.2 + 0.8 / (1.0 + np.exp(-lcc))
    C = 0.1 / (1.0 + np.exp(-lil))
    put("C_t", C)
    put("c1", -1.0 / (K - C))
    put("nr", -r)
    put("mc3", 2.0 * r / (K - C))
    put("sts", tgt_std[0])
    put("tmb", b4[0] * tgt_std[0] + tgt_mean[0])
    return w16, w32


def _prep_in_maps(inputs):
    w16, w32 = _pack_consts(inputs)
    x = np.asarray(inputs["inputs"], np.float32).reshape(N, 2)
    in_maps = []
    for c in range(NCORES):
        in_maps.append(
            {
                "wblk16": w16,
                "wblk32": w32,
                "x2": np.ascontiguousarray(
                    x[c * NLOC : (c + 1) * NLOC].T
                ).astype(np.float16),
            }
        )
    return in_maps


def _unshard(res_get):
    U = np.empty((N,), np.float32)
    F = np.empty((N,), np.float32)
    Ft = np.empty((N,), np.float32)
    for c in range(NCORES):
        o = res_get(c)
        U[c * NLOC : (c + 1) * NLOC] = o[0]
        F[c * NLOC : (c + 1) * NLOC] = o[1]
        Ft[c * NLOC : (c + 1) * NLOC] = o[2]
    shp = (B, S, 1)
    return U.reshape(shp), F.reshape(shp), Ft.reshape(shp)


def run(inputs, trace=False):
    nc = _get_nc()
    in_maps = _prep_in_maps(inputs)
    kw = {}
    if trace:
        kw["tmpdir"] = tempfile.mkdtemp(prefix="bassk_prof_")
    res = run_bass_kernel_spmd(
        nc, in_maps, core_ids=list(range(NCORES)), trace=trace, **kw
    )
    return _unshard(lambda c: res.results[c]["out"]), res


def kernel(**inputs):
    outs, _ = run(inputs, trace=False)
    return outs


# ---------------------------------------------------------------------------
# Dev-loop timing: persistent jitted executable (mirrors
# bass2jax.run_bass_via_pjrt's multi-core branch) so repeated executions
# reuse one compiled NEFF and can be timed back-to-back.
# ---------------------------------------------------------------------------
def _make_runner():
    if "runner" in _STATE:
        return _STATE["runner"]
    import jax
    from jax.experimental.shard_map import shard_map
    from jax.sharding import Mesh, PartitionSpec
    from concourse import bass2jax

    bass2jax.install_neuronx_cc_hook()
    nc = _get_nc()

    in_names, out_names, out_avals, zero_outs = [], [], [], []
    for alloc in nc.m.functions[0].allocations:
        if not isinstance(alloc, mybir.MemoryLocationSet):
            continue
        name = alloc.memorylocations[0].name
        if alloc.kind == "ExternalInput":
            if nc.partition_id_tensor is None or name != nc.partition_id_tensor.name:
                in_names.append(name)
        elif alloc.kind == "ExternalOutput":
            out_names.append(name)
            shape = tuple(alloc.tensor_shape)
            dtype = mybir.dt.np(alloc.dtype)
            out_avals.append(jax.core.ShapedArray(shape, dtype))
            zero_outs.append(np.zeros(shape, dtype))
    n_params = len(in_names)
    n_outs = len(out_avals)
    all_names = in_names + out_names
    if nc.partition_id_tensor is not None:
        all_names = all_names + [nc.partition_id_tensor.name]

    def _body(*args):
        operands = list(args)
        if nc.partition_id_tensor is not None:
            operands.append(bass2jax.partition_id_tensor())
        outs = bass2jax._bass_exec_p.bind(
            *operands,
            out_avals=tuple(out_avals),
            in_names=tuple(all_names),
            out_names=tuple(out_names),
            lowering_input_output_aliases=(),
            sim_require_finite=True,
            sim_require_nnan=True,
            nc=nc,
        )
        return tuple(outs)

    devices = jax.devices()[:NCORES]
    mesh = Mesh(np.asarray(devices), ("core",))
    donate = tuple(range(n_params, n_params + n_outs))
    sharded = jax.jit(
        shard_map(
            _body,
            mesh=mesh,
            in_specs=(PartitionSpec("core"),) * (n_params + n_outs),
            out_specs=(PartitionSpec("core"),) * n_outs,
            check_rep=False,
        ),
        donate_argnums=donate,
        keep_unused=True,
    )
    _STATE["runner"] = (sharded, in_names, out_names, out_avals, zero_outs)
    return _STATE["runner"]


def run_timed(inputs, iters=20):
    """Run via a persistent executable; return (outputs, per_iter_ns)."""
    import time as _time

    import jax

    sharded, in_names, out_names, out_avals, zero_outs = _make_runner()
    in_maps = _prep_in_maps(inputs)
    concat_in = [
        np.concatenate([np.asarray(in_maps[c][n]) for c in range(NCORES)], axis=0)
        for n in in_names
    ]
    dev_in = [jax.device_put(a) for a in concat_in]

    def zeros():
        return [
            np.zeros((NCORES * z.shape[0], *z.shape[1:]), z.dtype) for z in zero_outs
        ]

    # warmup (compiles on first call)
    outs = sharded(*dev_in, *zeros())
    jax.block_until_ready(outs)
    out_np = [np.asarray(o) for o in outs]

    zbufs = [zeros() for _ in range(iters)]
    t0 = _time.perf_counter()
    last = None
    for i in range(iters):
        last = sharded(*dev_in, *zbufs[i])
    jax.block_until_ready(last)
    t1 = _time.perf_counter()
    per_iter_ns = (t1 - t0) / iters * 1e9

    per_core = [
        {
            name: out_np[i].reshape(NCORES, *out_avals[i].shape)[c]
            for i, name in enumerate(out_names)
        }
        for c in range(NCORES)
    ]
    return _unshard(lambda c: per_core[c]["out"]), per_iter_ns


# revision 15
# speedup vs baseline: 1.3174x; 1.3174x over previous
"""Trainium2 Bass kernel for the CapacityNN PINN forward pass (v2).

Computes, for N = B*S collocation points x = (s, t):
  U   = MLP([s_norm, t_norm]) * tgt_std + tgt_mean
  F   = U_t  - G(U)             (G = Verhulst logistic growth term)
  F_t = U_tt - G'(U) * U_t
with U_t/U_tt computed exactly by forward-mode 2nd-order jet propagation
through the tanh MLP.

Sharding: pure data parallel over 8 NeuronCores (8192 points/core),
MLP weights + PDE scalars replicated (host-folded).

v2 layout/engine plan (from TimelineSim cost-model analysis):
  - all streams fp16 (DVE 2x tensor_tensor / 4x tensor_scalar modes)
  - [128, 1024]-wide PSUM tiles -> one Act/DVE op per stream per layer
  - elementwise jet algebra balanced across Act / DVE / Pool:
      Act : tanh, st=2*z1^2 (PSUM reads), 2 of 8 ee squares, final copies
      DVE : ee, dm=1-ee, tt=av*st, h2=dm*qt, ad0, 4 of 6 h1=dm*z1
      Pool: qt=z2-tt (PSUM read), 2 of 6 h1
  - all scalar prep + weight folding done on HOST; weights arrive as two
    pre-packed SBUF-image blocks (2 big DMAs instead of ~60 small ones)
  - tail transpose ([3,NLOC] -> [128,3*PPP]) streamed per-chunk via
    SBUF->SBUF DMAs overlapped with compute
"""

import os
import sys
import tempfile

import numpy as np

for _p in ("/opt/trn_rl_repo", "/root/.axon_site/_ro/trn_rl_repo"):
    if os.path.isdir(_p) and _p not in sys.path:
        sys.path.insert(0, _p)

import concourse.bass as bass
import concourse.bacc as bacc
import concourse.tile as tile
from concourse import mybir
from concourse.bass_utils import run_bass_kernel_spmd

AF = mybir.ActivationFunctionType
OP = mybir.AluOpType
F32 = mybir.dt.float32
F16 = mybir.dt.float16

NCORES = 8
B, S, H = 512, 128, 256
N = B * S                  # 65536 points
NLOC = N // NCORES         # 8192 points per core
CH = 1024                  # points per on-chip chunk
NCHUNK = NLOC // CH
PPP = NLOC // 128          # points per partition in the tail layout (64)
CPC = CH // 128            # tail cols per chunk (8)
SQRT2 = float(np.sqrt(2.0))

# ---- packed fp16 const block column map (must match _pack_w16) ----
# 21 [128,128] weight tiles (incl negid), then w0ts (rows 0-1), then w4 (2 cols)
_W16_TILES = []  # (name, col) in order
_c = 0
for _l in (1, 2, 3):
    for _kk in range(2):
        for _mm in range(2):
            _W16_TILES.append((f"wt{_l}_{_kk}{_mm}", _c))
            _c += 128
for _nm in ("wtw", "wtw2"):
    for _kk in range(2):
        for _mm in range(2):
            _W16_TILES.append((f"{_nm}_{_kk}{_mm}", _c))
            _c += 128
_W16_TILES.append(("negid", _c))
_c += 128
_W16_W0TS = _c          # [2, 256] at rows 0-1, cols [_c, _c+256)
_c += 256
_W16_W4 = _c            # [128, 2]: col kk = W4 half kk
_c += 2
W16COLS = _c

# ---- packed fp32 const block column map (must match _pack_w32) ----
# [128, W32COLS]: per-partition scalars and biases, one col each
_W32_NAMES = [
    "beta0_0", "beta0_1",
    "bl1_0", "bl1_1", "bl2_0", "bl2_1", "bl3_0", "bl3_1",
    "C_t", "c1", "nr", "mc3", "sts", "tmb",
]
W32COLS = len(_W32_NAMES)
_W32_IDX = {n: i for i, n in enumerate(_W32_NAMES)}


def _build():
    nc = bacc.Bacc(
        "TRN2",
        target_bir_lowering=False,
        debug=False,
        enable_asserts=False,
        num_devices=NCORES,
    )

    x2 = nc.dram_tensor("x2", [2, NLOC], F16, kind="ExternalInput").ap()
    wblk16 = nc.dram_tensor("wblk16", [128, W16COLS], F16, kind="ExternalInput").ap()
    wblk32 = nc.dram_tensor("wblk32", [128, W32COLS], F32, kind="ExternalInput").ap()
    out = nc.dram_tensor("out", [3, NLOC], F32, kind="ExternalOutput").ap()

    with tile.TileContext(nc) as tc:
        from contextlib import ExitStack

        with ExitStack() as ctx:
            const = ctx.enter_context(tc.tile_pool(name="const", bufs=1))
            sb = ctx.enter_context(tc.tile_pool(name="sb", bufs=1))
            ps = ctx.enter_context(tc.tile_pool(name="ps", bufs=1, space="PSUM"))

            # ---------- const loads: two big DMAs ----------
            w16 = const.tile([128, W16COLS], F16, name="w16")
            w32 = const.tile([128, W32COLS], F32, name="w32")
            nc.sync.dma_start(out=w16, in_=wblk16)
            nc.sync.dma_start(out=w32, in_=wblk32)

            wtile = {}
            for nm, col in _W16_TILES:
                wtile[nm] = w16[:, col : col + 128]
            w0ts = w16[0:2, _W16_W0TS : _W16_W0TS + 256]  # [2, 256]
            w4c = [w16[:, _W16_W4 + kk : _W16_W4 + kk + 1] for kk in range(2)]

            def sc(name):
                i = _W32_IDX[name]
                return w32[:, i : i + 1]

            beta0 = [sc("beta0_0"), sc("beta0_1")]
            bl = {l: [sc(f"bl{l}_0"), sc(f"bl{l}_1")] for l in (1, 2, 3)}
            C_t, c1, nr, mc3, sts, tmb = (
                sc("C_t"), sc("c1"), sc("nr"), sc("mc3"), sc("sts"), sc("tmb"),
            )

            # ---------- main loop: software-pipelined chunk PAIRS ----------
            # PE/Act/DVE/Pool execute their queues in order, so matmuls and
            # elementwise consumers of the two chunks in a pair are emitted
            # interleaved: while chunk A's tanh->ee->dm->h1 chain drains,
            # the PE runs chunk B's matmuls (keeps the PE p-state ramped).
            # oc[p, c*24 + b*3 + s] = stream-s output for point c*CH + b*128 + p
            oc = sb.tile([128, 3 * PPP], F32, name="oc")

            PSB = 4   # psum [128,1024] ring (2 banks each -> all 8 banks)
            st8 = {}  # per-chunk live tiles

            def new_stream(c, tag, bufs=3):
                return sb.tile([128, CH], F16, tag=tag, bufs=bufs, name=tag)

            def l0_mm(c):
                x2c = sb.tile([2, CH], F16, tag="x2c", bufs=2)
                nc.sync.dma_start(out=x2c, in_=x2[:, c * CH : (c + 1) * CH])
                pzs = []
                for m in range(2):
                    pz = ps.tile([128, CH], F32, tag="pz", bufs=PSB, name="pz0")
                    for g in range(2):
                        nc.tensor.matmul(
                            pz[:, g * 512 : (g + 1) * 512],
                            w0ts[:, m * 128 : (m + 1) * 128],
                            x2c[:, g * 512 : (g + 1) * 512],
                            start=True,
                            stop=True,
                        )
                    pzs.append(pz)
                st8[c] = {"pz": pzs}

            def l0_cons(c):
                s = st8[c]
                Hv, H1, H2 = [None] * 2, [None] * 2, [None] * 2
                for m in range(2):
                    av = new_stream(c, f"hv{m}")
                    ee = new_stream(c, f"ee{m}", 2)
                    dm = new_stream(c, f"dm{m}", 2)
                    ad = new_stream(c, f"ad{m}", 2)
                    nc.scalar.activation(av, s["pz"][m], AF.Tanh, beta0[m])
                    nc.vector.tensor_tensor(ee, av, av, OP.mult)
                    nc.vector.tensor_scalar(dm, ee, -1.0, 1.0, OP.mult, OP.add)
                    nc.gpsimd.tensor_tensor(ad, av, dm, OP.mult)
                    Hv[m], H1[m], H2[m] = av, dm, ad
                s["Hv"], s["H1"], s["H2"] = Hv, H1, H2

            def prim_mm(c, l):
                s = st8[c]
                s["pz"] = []
                for m in range(2):
                    pz = ps.tile([128, CH], F32, tag="pz", bufs=PSB, name="pzv")
                    for g in range(2):
                        for kk in range(2):
                            nc.tensor.matmul(
                                pz[:, g * 512 : (g + 1) * 512],
                                wtile[f"wt{l}_{kk}{m}"],
                                s["Hv"][kk][:, g * 512 : (g + 1) * 512],
                                start=(kk == 0),
                                stop=(kk == 1),
                            )
                    s["pz"].append(pz)

            def tanh_emit(c, l):
                s = st8[c]
                s["nHv"] = []
                for m in range(2):
                    av = new_stream(c, f"hv{m}")
                    nc.scalar.activation(av, s["pz"][m], AF.Tanh, bl[l][m])
                    s["nHv"].append(av)

            def d1_mm(c, l):
                s = st8[c]
                s["pz1"] = []
                for m in range(2):
                    pz1 = ps.tile([128, CH], F32, tag="pz", bufs=PSB, name="pz1")
                    w1nm = f"wtw_{{}}{m}" if l == 1 else f"wt{l}_{{}}{m}"
                    for g in range(2):
                        for kk in range(2):
                            nc.tensor.matmul(
                                pz1[:, g * 512 : (g + 1) * 512],
                                wtile[w1nm.format(kk)],
                                s["H1"][kk][:, g * 512 : (g + 1) * 512],
                                start=(kk == 0),
                                stop=(kk == 1),
                            )
                    s["pz1"].append(pz1)

            def st_emit(c, l):
                s = st8[c]
                s["st"] = []
                for m in range(2):
                    st = new_stream(c, f"st{m}", 2)
                    nc.scalar.activation(st, s["pz1"][m], AF.Square, 0.0, SQRT2)
                    s["st"].append(st)

            def eedm_emit(c, l):
                s = st8[c]
                s["nDm"] = []
                for m in range(2):
                    ee = new_stream(c, f"ee{m}", 2)
                    dm = new_stream(c, f"dm{m}", 2)
                    if l == 1 or (l == 2 and m == 0):
                        nc.scalar.activation(ee, s["nHv"][m], AF.Square)
                    else:
                        nc.vector.tensor_tensor(ee, s["nHv"][m], s["nHv"][m], OP.mult)
                    nc.vector.tensor_scalar(dm, ee, -1.0, 1.0, OP.mult, OP.add)
                    s["nDm"].append(dm)

            def h1tt_emit(c, l):
                s = st8[c]
                s["nH1"], s["Tt"] = [], []
                for m in range(2):
                    h1t = new_stream(c, f"h1{m}")
                    tt = new_stream(c, f"tt{m}", 2)
                    nc.vector.tensor_tensor(h1t, s["nDm"][m], s["pz1"][m], OP.mult)
                    nc.gpsimd.tensor_tensor(tt, s["nHv"][m], s["st"][m], OP.mult)
                    s["nH1"].append(h1t)
                    s["Tt"].append(tt)

            def d2_mm(c, l):
                s = st8[c]
                s["pz2"] = []
                for m in range(2):
                    pz2 = ps.tile([128, CH], F32, tag="pz", bufs=PSB, name="pz2")
                    w2nm = f"wtw2_{{}}{m}" if l == 1 else f"wt{l}_{{}}{m}"
                    for g in range(2):
                        for kk in range(2):
                            nc.tensor.matmul(
                                pz2[:, g * 512 : (g + 1) * 512],
                                wtile[w2nm.format(kk)],
                                s["H2"][kk][:, g * 512 : (g + 1) * 512],
                                start=(kk == 0),
                                stop=False,
                            )
                    s["pz2"].append(pz2)

            def negid_mm(c, l):
                # qt = z2 - tt: accumulate -I @ tt into the z2 psum group
                s = st8[c]
                for m in range(2):
                    for g in range(2):
                        nc.tensor.matmul(
                            s["pz2"][m][:, g * 512 : (g + 1) * 512],
                            wtile["negid"],
                            s["Tt"][m][:, g * 512 : (g + 1) * 512],
                            start=False,
                            stop=True,
                        )

            def h2_emit(c, l):
                s = st8[c]
                s["nH2"] = []
                for m in range(2):
                    h2t = new_stream(c, f"h2{m}")
                    nc.vector.tensor_tensor(h2t, s["nDm"][m], s["pz2"][m], OP.mult)
                    s["nH2"].append(h2t)

            def layer_rotate(c):
                s = st8[c]
                s["Hv"], s["H1"], s["H2"] = s["nHv"], s["nH1"], s["nH2"]

            def final_mm(c):
                s = st8[c]
                pblk = ps.tile([128, 3 * CPC], F32, tag="pz", bufs=PSB, name="pblk")
                for b in range(CPC):
                    for s_idx, stream in enumerate((s["Hv"], s["H1"], s["H2"])):
                        for kk in range(2):
                            nc.tensor.matmul(
                                pblk[:, b * 3 + s_idx : b * 3 + s_idx + 1],
                                stream[kk][:, b * 128 : (b + 1) * 128],
                                w4c[kk],
                                start=(kk == 0),
                                stop=(kk == 1),
                            )
                s["pblk"] = pblk

            def final_copy(c):
                nc.scalar.copy(
                    oc[:, c * 3 * CPC : (c + 1) * 3 * CPC], st8[c]["pblk"]
                )

            for c0 in range(0, NCHUNK, 2):
                pair = (c0, c0 + 1)
                for c in pair:
                    l0_mm(c)
                for c in pair:
                    l0_cons(c)
                for l in (1, 2, 3):
                    for c in pair:
                        prim_mm(c, l)
                    for c in pair:
                        tanh_emit(c, l)
                    for c in pair:
                        d1_mm(c, l)
                    for c in pair:
                        st_emit(c, l)
                    for c in pair:
                        eedm_emit(c, l)
                    for c in pair:
                        h1tt_emit(c, l)
                    for c in pair:
                        d2_mm(c, l)
                    for c in pair:
                        negid_mm(c, l)
                    for c in pair:
                        h2_emit(c, l)
                    for c in pair:
                        layer_rotate(c)
                for c in pair:
                    final_mm(c)
                for c in pair:
                    final_copy(c)

            # ----- tail (once): PDE algebra on [128, 3*PPP], strided views -----
            oc2 = sb.tile([128, 3 * PPP], F32, name="oc2")

            yv, yt, ytt = (oc[:, s : 3 * PPP : 3] for s in range(3))
            U, Fo, Ft = (oc2[:, s : 3 * PPP : 3] for s in range(3))

            def tl(name):
                return sb.tile([128, PPP], F32, name=name)

            ut, utt, vv, v2, w1, q1, t1 = (
                tl("ut"), tl("utt"), tl("vv"), tl("v2"), tl("w1"), tl("q1"), tl("t1"),
            )
            nc.vector.tensor_scalar(U, yv, sts, tmb, OP.mult, OP.add)
            nc.vector.tensor_scalar(ut, yt, sts, None, OP.mult)
            nc.vector.tensor_scalar(utt, ytt, sts, None, OP.mult)
            nc.vector.tensor_scalar(vv, U, C_t, None, OP.subtract)
            nc.vector.tensor_tensor(v2, vv, vv, OP.mult)
            nc.vector.scalar_tensor_tensor(w1, v2, c1, vv, OP.mult, OP.add)
            nc.vector.scalar_tensor_tensor(Fo, w1, nr, ut, OP.mult, OP.add)
            nc.vector.tensor_tensor(q1, vv, ut, OP.mult)
            nc.vector.scalar_tensor_tensor(t1, ut, nr, utt, OP.mult, OP.add)
            nc.vector.scalar_tensor_tensor(Ft, q1, mc3, t1, OP.mult, OP.add)

            # out[s, c*CH + b*128 + p] = oc2[p, c*3*CPC + b*3 + s]
            for s_idx in range(3):
                nc.sync.dma_start(
                    out=bass.AP(
                        out.tensor,
                        s_idx * NLOC,
                        [[1, 128], [CH, NCHUNK], [128, CPC]],
                    ),
                    in_=bass.AP(
                        oc2.tensor,
                        oc2.offset + s_idx,
                        [list(oc2.ap[0]), [3 * CPC, NCHUNK], [3, CPC]],
                    ),
                )

    nc.compile()
    return nc


_STATE = {}


def _get_nc():
    if "nc" not in _STATE:
        _STATE["nc"] = _build()
    return _STATE["nc"]


def _pack_consts(inputs):
    f = np.float32

    def arr(k):
        return np.ascontiguousarray(np.asarray(inputs[k], f))

    W0, b0 = arr("W0"), arr("b0")
    Ws = {1: arr("W1"), 2: arr("W2"), 3: arr("W3")}
    bs = {1: arr("b1"), 2: arr("b2"), 3: arr("b3")}
    W4, b4 = arr("W4").reshape(1, H), arr("b4").reshape(1)
    in_mean, in_std = arr("in_mean"), arr("in_std")
    tgt_mean, tgt_std = arr("tgt_mean"), arr("tgt_std")
    lgr = float(arr("log_growth_rate").reshape(-1)[0])
    lcc = float(arr("log_carrying_capacity").reshape(-1)[0])
    lil = float(arr("log_initial_loss").reshape(-1)[0])

    # fp16 block
    w16 = np.zeros((128, W16COLS), np.float16)
    for nm, col in _W16_TILES:
        if nm == "negid":
            tilev = -np.eye(128, dtype=np.float32)
        elif nm.startswith("wtw"):
            base, km = nm.rsplit("_", 1)
            kk, mm = int(km[0]), int(km[1])
            if base == "wtw":
                Wf = (Ws[1] * W0[:, 1][None, :]).T
            else:
                Wf = (Ws[1] * (-2.0 * W0[:, 1] ** 2)[None, :]).T
            tilev = Wf[kk * 128 : (kk + 1) * 128, mm * 128 : (mm + 1) * 128]
        else:
            l, km = nm[2:].split("_")
            l, kk, mm = int(l), int(km[0]), int(km[1])
            Wt = Ws[l].T  # [in, out]
            tilev = Wt[kk * 128 : (kk + 1) * 128, mm * 128 : (mm + 1) * 128]
        w16[:, col : col + 128] = tilev.astype(np.float16)
    # w0ts: W0.T rows scaled by 1/(std+eps)
    w0ts = (W0.T / (in_std[:, None] + 1e-8)).astype(np.float16)  # [2, H]
    w16[0:2, _W16_W0TS : _W16_W0TS + 256] = w0ts
    # w4 halves as [128, 2]; negid handled in the tile loop below
    for kk in range(2):
        w16[:, _W16_W4 + kk] = W4[0, kk * 128 : (kk + 1) * 128].astype(np.float16)

    # fp32 block
    w32 = np.zeros((128, W32COLS), np.float32)

    def put(name, vec):
        w32[:, _W32_IDX[name]] = vec

    m0i = in_mean[0] / (in_std[0] + 1e-8)
    m1i = in_mean[1] / (in_std[1] + 1e-8)
    u = W0[:, 0] * m0i + W0[:, 1] * m1i
    beta0 = b0 - u
    put("beta0_0", beta0[0:128])
    put("beta0_1", beta0[128:256])
    for l in (1, 2, 3):
        put(f"bl{l}_0", bs[l][0:128])
        put(f"bl{l}_1", bs[l][128:256])
    r = np.exp(-lgr)
    K = 0.2 + 0.8 / (1.0 + np.exp(-lcc))
    C = 0.1 / (1.0 + np.exp(-lil))
    put("C_t", C)
    put("c1", -1.0 / (K - C))
    put("nr", -r)
    put("mc3", 2.0 * r / (K - C))
    put("sts", tgt_std[0])
    put("tmb", b4[0] * tgt_std[0] + tgt_mean[0])
    return w16, w32


def _prep_in_maps(inputs):
    w16, w32 = _pack_consts(inputs)
    x = np.asarray(inputs["inputs"], np.float32).reshape(N, 2)
    in_maps = []
    for c in range(NCORES):
        in_maps.append(
            {
                "wblk16": w16,
                "wblk32": w32,
                "x2": np.ascontiguousarray(
                    x[c * NLOC : (c + 1) * NLOC].T
                ).astype(np.float16),
            }
        )
    return in_maps


def _unshard(res_get):
    U = np.empty((N,), np.float32)
    F = np.empty((N,), np.float32)
    Ft = np.empty((N,), np.float32)
    for c in range(NCORES):
        o = res_get(c)
        U[c * NLOC : (c + 1) * NLOC] = o[0]
        F[c * NLOC : (c + 1) * NLOC] = o[1]
        Ft[c * NLOC : (c + 1) * NLOC] = o[2]
    shp = (B, S, 1)
    return U.reshape(shp), F.reshape(shp), Ft.reshape(shp)


def run(inputs, trace=False):
    nc = _get_nc()
    in_maps = _prep_in_maps(inputs)
    kw = {}
    if trace:
        kw["tmpdir"] = tempfile.mkdtemp(prefix="bassk_prof_")
    res = run_bass_kernel_spmd(
        nc, in_maps, core_ids=list(range(NCORES)), trace=trace, **kw
    )
    return _unshard(lambda c: res.results[c]["out"]), res


def kernel(**inputs):
    outs, _ = run(inputs, trace=False)
    return outs


# ---------------------------------------------------------------------------
# Dev-loop timing: persistent jitted executable (mirrors
# bass2jax.run_bass_via_pjrt's multi-core branch) so repeated executions
# reuse one compiled NEFF and can be timed back-to-back.
# ---------------------------------------------------------------------------
def _make_runner():
    if "runner" in _STATE:
        return _STATE["runner"]
    import jax
    from jax.experimental.shard_map import shard_map
    from jax.sharding import Mesh, PartitionSpec
    from concourse import bass2jax

    bass2jax.install_neuronx_cc_hook()
    nc = _get_nc()

    in_names, out_names, out_avals, zero_outs = [], [], [], []
    for alloc in nc.m.functions[0].allocations:
        if not isinstance(alloc, mybir.MemoryLocationSet):
            continue
        name = alloc.memorylocations[0].name
        if alloc.kind == "ExternalInput":
            if nc.partition_id_tensor is None or name != nc.partition_id_tensor.name:
                in_names.append(name)
        elif alloc.kind == "ExternalOutput":
            out_names.append(name)
            shape = tuple(alloc.tensor_shape)
            dtype = mybir.dt.np(alloc.dtype)
            out_avals.append(jax.core.ShapedArray(shape, dtype))
            zero_outs.append(np.zeros(shape, dtype))
    n_params = len(in_names)
    n_outs = len(out_avals)
    all_names = in_names + out_names
    if nc.partition_id_tensor is not None:
        all_names = all_names + [nc.partition_id_tensor.name]

    def _body(*args):
        operands = list(args)
        if nc.partition_id_tensor is not None:
            operands.append(bass2jax.partition_id_tensor())
        outs = bass2jax._bass_exec_p.bind(
            *operands,
            out_avals=tuple(out_avals),
            in_names=tuple(all_names),
            out_names=tuple(out_names),
            lowering_input_output_aliases=(),
            sim_require_finite=True,
            sim_require_nnan=True,
            nc=nc,
        )
        return tuple(outs)

    devices = jax.devices()[:NCORES]
    mesh = Mesh(np.asarray(devices), ("core",))
    donate = tuple(range(n_params, n_params + n_outs))
    sharded = jax.jit(
        shard_map(
            _body,
            mesh=mesh,
            in_specs=(PartitionSpec("core"),) * (n_params + n_outs),
            out_specs=(PartitionSpec("core"),) * n_outs,
            check_rep=False,
        ),
        donate_argnums=donate,
        keep_unused=True,
    )
    _STATE["runner"] = (sharded, in_names, out_names, out_avals, zero_outs)
    return _STATE["runner"]


def run_timed(inputs, iters=20):
    """Run via a persistent executable; return (outputs, per_iter_ns)."""
    import time as _time

    import jax

    sharded, in_names, out_names, out_avals, zero_outs = _make_runner()
    in_maps = _prep_in_maps(inputs)
    concat_in = [
        np.concatenate([np.asarray(in_maps[c][n]) for c in range(NCORES)], axis=0)
        for n in in_names
    ]
    dev_in = [jax.device_put(a) for a in concat_in]

    def zeros():
        return [
            np.zeros((NCORES * z.shape[0], *z.shape[1:]), z.dtype) for z in zero_outs
        ]

    # warmup (compiles on first call)
    outs = sharded(*dev_in, *zeros())
    jax.block_until_ready(outs)
    out_np = [np.asarray(o) for o in outs]

    zbufs = [zeros() for _ in range(iters)]
    t0 = _time.perf_counter()
    last = None
    for i in range(iters):
        last = sharded(*dev_in, *zbufs[i])
    jax.block_until_ready(last)
    t1 = _time.perf_counter()
    per_iter_ns = (t1 - t0) / iters * 1e9

    per_core = [
        {
            name: out_np[i].reshape(NCORES, *out_avals[i].shape)[c]
            for i, name in enumerate(out_names)
        }
        for c in range(NCORES)
    ]
    return _unshard(lambda c: per_core[c]["out"]), per_iter_ns


# revision 21
# speedup vs baseline: 1.3565x; 1.0297x over previous
"""Trainium2 Bass kernel for the CapacityNN PINN forward pass (v2).

Computes, for N = B*S collocation points x = (s, t):
  U   = MLP([s_norm, t_norm]) * tgt_std + tgt_mean
  F   = U_t  - G(U)             (G = Verhulst logistic growth term)
  F_t = U_tt - G'(U) * U_t
with U_t/U_tt computed exactly by forward-mode 2nd-order jet propagation
through the tanh MLP.

Sharding: pure data parallel over 8 NeuronCores (8192 points/core),
MLP weights + PDE scalars replicated (host-folded).

v2 layout/engine plan (from TimelineSim cost-model analysis):
  - all streams fp16 (DVE 2x tensor_tensor / 4x tensor_scalar modes)
  - [128, 1024]-wide PSUM tiles -> one Act/DVE op per stream per layer
  - elementwise jet algebra balanced across Act / DVE / Pool:
      Act : tanh, st=2*z1^2 (PSUM reads), 2 of 8 ee squares, final copies
      DVE : ee, dm=1-ee, tt=av*st, h2=dm*qt, ad0, 4 of 6 h1=dm*z1
      Pool: qt=z2-tt (PSUM read), 2 of 6 h1
  - all scalar prep + weight folding done on HOST; weights arrive as two
    pre-packed SBUF-image blocks (2 big DMAs instead of ~60 small ones)
  - tail transpose ([3,NLOC] -> [128,3*PPP]) streamed per-chunk via
    SBUF->SBUF DMAs overlapped with compute
"""

import os
import sys
import tempfile

import numpy as np

for _p in ("/opt/trn_rl_repo", "/root/.axon_site/_ro/trn_rl_repo"):
    if os.path.isdir(_p) and _p not in sys.path:
        sys.path.insert(0, _p)

import concourse.bass as bass
import concourse.bacc as bacc
import concourse.tile as tile
from concourse import mybir
from concourse.bass_utils import run_bass_kernel_spmd

AF = mybir.ActivationFunctionType
OP = mybir.AluOpType
F32 = mybir.dt.float32
F16 = mybir.dt.float16

NCORES = 8
B, S, H = 512, 128, 256
N = B * S                  # 65536 points
NLOC = N // NCORES         # 8192 points per core
CH = 1024                  # points per on-chip chunk
NCHUNK = NLOC // CH
PPP = NLOC // 128          # points per partition in the tail layout (64)
CPC = CH // 128            # tail cols per chunk (8)
SQRT2 = float(np.sqrt(2.0))

# ---- packed fp16 const block column map (must match _pack_w16) ----
# 21 [128,128] weight tiles (incl negid), then w0ts (rows 0-1), then w4 (2 cols)
_W16_TILES = []  # (name, col) in order
_c = 0
for _l in (1, 2, 3):
    for _kk in range(2):
        for _mm in range(2):
            _W16_TILES.append((f"wt{_l}_{_kk}{_mm}", _c))
            _c += 128
for _nm in ("wtw", "wtw2"):
    for _kk in range(2):
        for _mm in range(2):
            _W16_TILES.append((f"{_nm}_{_kk}{_mm}", _c))
            _c += 128
_W16_TILES.append(("negid", _c))
_c += 128
_W16_W0TS = _c          # [2, 256] at rows 0-1, cols [_c, _c+256)
_c += 256
_W16_W4 = _c            # [128, 2]: col kk = W4 half kk
_c += 2
W16COLS = _c

# ---- packed fp32 const block column map (must match _pack_w32) ----
# [128, W32COLS]: per-partition scalars and biases, one col each
_W32_NAMES = [
    "beta0_0", "beta0_1",
    "bl1_0", "bl1_1", "bl2_0", "bl2_1", "bl3_0", "bl3_1",
    "C_t", "c1", "nr", "mc3", "sts", "tmb",
]
W32COLS = len(_W32_NAMES)
_W32_IDX = {n: i for i, n in enumerate(_W32_NAMES)}


def _build():
    nc = bacc.Bacc(
        "TRN2",
        target_bir_lowering=False,
        debug=False,
        enable_asserts=False,
        num_devices=NCORES,
    )

    x2 = nc.dram_tensor("x2", [2, NLOC], F16, kind="ExternalInput").ap()
    wblk16 = nc.dram_tensor("wblk16", [128, W16COLS], F16, kind="ExternalInput").ap()
    wblk32 = nc.dram_tensor("wblk32", [128, W32COLS], F32, kind="ExternalInput").ap()
    out = nc.dram_tensor("out", [3, NLOC], F32, kind="ExternalOutput").ap()

    with tile.TileContext(nc) as tc:
        from contextlib import ExitStack

        with ExitStack() as ctx:
            const = ctx.enter_context(tc.tile_pool(name="const", bufs=1))
            sb = ctx.enter_context(tc.tile_pool(name="sb", bufs=1))
            ps = ctx.enter_context(tc.tile_pool(name="ps", bufs=1, space="PSUM"))

            # ---------- const loads ----------
            # split so layer-0's weights (w0ts, at the tail of the block)
            # arrive before the bulk of the hidden-layer tiles
            w16 = const.tile([128, W16COLS], F16, name="w16")
            w32 = const.tile([128, W32COLS], F32, name="w32")
            _SPLIT = _W16_TILES[-1][1]  # negid col: negid+w0ts+w4 in first DMA
            nc.sync.dma_start(out=w16[:, _SPLIT:], in_=wblk16[:, _SPLIT:])
            nc.sync.dma_start(out=w32, in_=wblk32)
            # bulk hidden-layer weights stream in behind the first x2c loads
            _wbulk = [False]

            def load_wbulk():
                if not _wbulk[0]:
                    _wbulk[0] = True
                    nc.sync.dma_start(out=w16[:, :_SPLIT], in_=wblk16[:, :_SPLIT])

            wtile = {}
            for nm, col in _W16_TILES:
                wtile[nm] = w16[:, col : col + 128]
            w0ts = w16[0:2, _W16_W0TS : _W16_W0TS + 256]  # [2, 256]
            w4c = [w16[:, _W16_W4 + kk : _W16_W4 + kk + 1] for kk in range(2)]

            def sc(name):
                i = _W32_IDX[name]
                return w32[:, i : i + 1]

            beta0 = [sc("beta0_0"), sc("beta0_1")]
            bl = {l: [sc(f"bl{l}_0"), sc(f"bl{l}_1")] for l in (1, 2, 3)}
            C_t, c1, nr, mc3, sts, tmb = (
                sc("C_t"), sc("c1"), sc("nr"), sc("mc3"), sc("sts"), sc("tmb"),
            )

            # ---------- main loop: software-pipelined chunk PAIRS ----------
            # PE/Act/DVE/Pool execute their queues in order, so matmuls and
            # elementwise consumers of the two chunks in a pair are emitted
            # interleaved: while chunk A's tanh->ee->dm->h1 chain drains,
            # the PE runs chunk B's matmuls (keeps the PE p-state ramped).
            # oc[p, c*24 + b*3 + s] = stream-s output for point c*CH + b*128 + p
            oc = sb.tile([128, 3 * PPP], F32, name="oc")
            oc2 = sb.tile([128, 3 * PPP], F32, name="oc2")

            PSB = 4   # psum [128,1024] ring (2 banks each -> all 8 banks)
            st8 = {}  # per-chunk live tiles

            def new_stream(c, tag, bufs=3):
                return sb.tile([128, CH], F16, tag=tag, bufs=bufs, name=tag)

            def l0_mm(c):
                x2c = sb.tile([2, CH], F16, tag="x2c", bufs=2)
                nc.sync.dma_start(out=x2c, in_=x2[:, c * CH : (c + 1) * CH])
                pzs = []
                for m in range(2):
                    pz = ps.tile([128, CH], F32, tag="pz", bufs=PSB, name="pz0")
                    for g in range(2):
                        nc.tensor.matmul(
                            pz[:, g * 512 : (g + 1) * 512],
                            w0ts[:, m * 128 : (m + 1) * 128],
                            x2c[:, g * 512 : (g + 1) * 512],
                            start=True,
                            stop=True,
                        )
                    pzs.append(pz)
                st8[c] = {"pz": pzs}

            def l0_cons(c):
                s = st8[c]
                Hv, H1, H2 = [None] * 2, [None] * 2, [None] * 2
                for m in range(2):
                    av = new_stream(c, f"hv{m}")
                    ee = new_stream(c, f"ee{m}", 2)
                    dm = new_stream(c, f"dm{m}", 2)
                    ad = new_stream(c, f"ad{m}", 2)
                    nc.scalar.activation(av, s["pz"][m], AF.Tanh, beta0[m])
                    nc.vector.tensor_tensor(ee, av, av, OP.mult)
                    nc.vector.tensor_scalar(dm, ee, -1.0, 1.0, OP.mult, OP.add)
                    nc.gpsimd.tensor_tensor(ad, av, dm, OP.mult)
                    Hv[m], H1[m], H2[m] = av, dm, ad
                s["Hv"], s["H1"], s["H2"] = Hv, H1, H2

            def prim_mm(c, l):
                s = st8[c]
                s["pz"] = []
                for m in range(2):
                    pz = ps.tile([128, CH], F32, tag="pz", bufs=PSB, name="pzv")
                    for g in range(2):
                        for kk in range(2):
                            nc.tensor.matmul(
                                pz[:, g * 512 : (g + 1) * 512],
                                wtile[f"wt{l}_{kk}{m}"],
                                s["Hv"][kk][:, g * 512 : (g + 1) * 512],
                                start=(kk == 0),
                                stop=(kk == 1),
                            )
                    s["pz"].append(pz)

            def tanh_emit(c, l):
                s = st8[c]
                s["nHv"] = []
                for m in range(2):
                    av = new_stream(c, f"hv{m}")
                    nc.scalar.activation(av, s["pz"][m], AF.Tanh, bl[l][m])
                    s["nHv"].append(av)

            def d1_mm(c, l):
                s = st8[c]
                s["pz1"] = []
                for m in range(2):
                    pz1 = ps.tile([128, CH], F32, tag="pz", bufs=PSB, name="pz1")
                    w1nm = f"wtw_{{}}{m}" if l == 1 else f"wt{l}_{{}}{m}"
                    for g in range(2):
                        for kk in range(2):
                            nc.tensor.matmul(
                                pz1[:, g * 512 : (g + 1) * 512],
                                wtile[w1nm.format(kk)],
                                s["H1"][kk][:, g * 512 : (g + 1) * 512],
                                start=(kk == 0),
                                stop=(kk == 1),
                            )
                    s["pz1"].append(pz1)

            def st_emit(c, l):
                s = st8[c]
                s["st"] = []
                for m in range(2):
                    st = new_stream(c, f"st{m}", 2)
                    nc.scalar.activation(st, s["pz1"][m], AF.Square, 0.0, SQRT2)
                    s["st"].append(st)

            def eedm_emit(c, l):
                s = st8[c]
                s["nDm"] = []
                for m in range(2):
                    ee = new_stream(c, f"ee{m}", 2)
                    dm = new_stream(c, f"dm{m}", 2)
                    if l == 1 or (l == 2 and m == 0):
                        nc.scalar.activation(ee, s["nHv"][m], AF.Square)
                    else:
                        nc.vector.tensor_tensor(ee, s["nHv"][m], s["nHv"][m], OP.mult)
                    nc.vector.tensor_scalar(dm, ee, -1.0, 1.0, OP.mult, OP.add)
                    s["nDm"].append(dm)

            def h1tt_emit(c, l):
                s = st8[c]
                s["nH1"], s["Tt"] = [], []
                for m in range(2):
                    h1t = new_stream(c, f"h1{m}")
                    tt = new_stream(c, f"tt{m}", 2)
                    nc.vector.tensor_tensor(h1t, s["nDm"][m], s["pz1"][m], OP.mult)
                    nc.gpsimd.tensor_tensor(tt, s["nHv"][m], s["st"][m], OP.mult)
                    s["nH1"].append(h1t)
                    s["Tt"].append(tt)

            def d2_mm(c, l):
                s = st8[c]
                s["pz2"] = []
                for m in range(2):
                    pz2 = ps.tile([128, CH], F32, tag="pz", bufs=PSB, name="pz2")
                    w2nm = f"wtw2_{{}}{m}" if l == 1 else f"wt{l}_{{}}{m}"
                    for g in range(2):
                        for kk in range(2):
                            nc.tensor.matmul(
                                pz2[:, g * 512 : (g + 1) * 512],
                                wtile[w2nm.format(kk)],
                                s["H2"][kk][:, g * 512 : (g + 1) * 512],
                                start=(kk == 0),
                                stop=False,
                            )
                    s["pz2"].append(pz2)

            def negid_mm(c, l):
                # qt = z2 - tt: accumulate -I @ tt into the z2 psum group
                s = st8[c]
                for m in range(2):
                    for g in range(2):
                        nc.tensor.matmul(
                            s["pz2"][m][:, g * 512 : (g + 1) * 512],
                            wtile["negid"],
                            s["Tt"][m][:, g * 512 : (g + 1) * 512],
                            start=False,
                            stop=True,
                        )

            def h2_emit(c, l):
                s = st8[c]
                s["nH2"] = []
                for m in range(2):
                    h2t = new_stream(c, f"h2{m}")
                    nc.vector.tensor_tensor(h2t, s["nDm"][m], s["pz2"][m], OP.mult)
                    s["nH2"].append(h2t)

            def layer_rotate(c):
                s = st8[c]
                s["Hv"], s["H1"], s["H2"] = s["nHv"], s["nH1"], s["nH2"]

            def final_mm(c):
                s = st8[c]
                pblk = ps.tile([128, 3 * CPC], F32, tag="pz", bufs=PSB, name="pblk")
                for b in range(CPC):
                    for s_idx, stream in enumerate((s["Hv"], s["H1"], s["H2"])):
                        for kk in range(2):
                            nc.tensor.matmul(
                                pblk[:, b * 3 + s_idx : b * 3 + s_idx + 1],
                                stream[kk][:, b * 128 : (b + 1) * 128],
                                w4c[kk],
                                start=(kk == 0),
                                stop=(kk == 1),
                            )
                s["pblk"] = pblk

            def final_copy(c):
                nc.scalar.copy(
                    oc[:, c * 3 * CPC : (c + 1) * 3 * CPC], st8[c]["pblk"]
                )

            def pair_pde(c0):
                # PDE algebra for this pair's [128, 48] slice of oc, on Pool
                # (Pool has slack; keeps the end-of-kernel tail tiny), then
                # the output DMAs for these two chunks.
                lo = c0 * 3 * CPC
                n = 2 * 3 * CPC
                osl = slice(lo, lo + n)
                ocp = oc[:, osl]
                ocq = oc2[:, osl]
                yv = ocp[:, 0:n:3]
                yt = ocp[:, 1:n:3]
                ytt = ocp[:, 2:n:3]
                U = ocq[:, 0:n:3]
                Fo = ocq[:, 1:n:3]
                Ft = ocq[:, 2:n:3]
                k = 2 * CPC

                def tl(name):
                    return sb.tile([128, k], F32, tag=name, bufs=2, name=name)

                ut, utt, vv, v2, w1, q1, t1 = (
                    tl("ut"), tl("utt"), tl("vv"), tl("v2"),
                    tl("w1"), tl("q1"), tl("t1"),
                )
                ve = nc.vector
                ve.tensor_scalar(U, yv, sts, tmb, OP.mult, OP.add)
                ve.tensor_scalar(ut, yt, sts, None, OP.mult)
                ve.tensor_scalar(utt, ytt, sts, None, OP.mult)
                ve.tensor_scalar(vv, U, C_t, None, OP.subtract)
                nc.gpsimd.tensor_tensor(v2, vv, vv, OP.mult)
                ve.scalar_tensor_tensor(w1, v2, c1, vv, OP.mult, OP.add)
                ve.scalar_tensor_tensor(Fo, w1, nr, ut, OP.mult, OP.add)
                nc.gpsimd.tensor_tensor(q1, vv, ut, OP.mult)
                ve.scalar_tensor_tensor(t1, ut, nr, utt, OP.mult, OP.add)
                ve.scalar_tensor_tensor(Ft, q1, mc3, t1, OP.mult, OP.add)
                # out[s, c*CH + b*128 + p] = oc2[p, c*3*CPC + b*3 + s]
                for s_idx in range(3):
                    nc.sync.dma_start(
                        out=bass.AP(
                            out.tensor,
                            s_idx * NLOC + c0 * CH,
                            [[1, 128], [CH, 2], [128, CPC]],
                        ),
                        in_=bass.AP(
                            oc2.tensor,
                            oc2.offset + lo + s_idx,
                            [list(oc2.ap[0]), [3 * CPC, 2], [3, CPC]],
                        ),
                    )

            for c0 in range(0, NCHUNK, 2):
                pair = (c0, c0 + 1)
                for c in pair:
                    l0_mm(c)
                load_wbulk()
                for c in pair:
                    l0_cons(c)
                for l in (1, 2, 3):
                    for c in pair:
                        prim_mm(c, l)
                    for c in pair:
                        tanh_emit(c, l)
                    for c in pair:
                        d1_mm(c, l)
                    for c in pair:
                        st_emit(c, l)
                    for c in pair:
                        eedm_emit(c, l)
                    for c in pair:
                        h1tt_emit(c, l)
                    last = l == 3
                    for c in pair:
                        d2_mm(c, l)
                        negid_mm(c, l)
                        h2_emit(c, l)
                        layer_rotate(c)
                        if last:
                            final_mm(c)
                    if last:
                        for c in pair:
                            final_copy(c)
                if c0 > 0:
                    pair_pde(c0 - 2)
            pair_pde(NCHUNK - 2)

    nc.compile()
    return nc


_STATE = {}


def _get_nc():
    if "nc" not in _STATE:
        _STATE["nc"] = _build()
    return _STATE["nc"]


def _pack_consts(inputs):
    f = np.float32

    def arr(k):
        return np.ascontiguousarray(np.asarray(inputs[k], f))

    W0, b0 = arr("W0"), arr("b0")
    Ws = {1: arr("W1"), 2: arr("W2"), 3: arr("W3")}
    bs = {1: arr("b1"), 2: arr("b2"), 3: arr("b3")}
    W4, b4 = arr("W4").reshape(1, H), arr("b4").reshape(1)
    in_mean, in_std = arr("in_mean"), arr("in_std")
    tgt_mean, tgt_std = arr("tgt_mean"), arr("tgt_std")
    lgr = float(arr("log_growth_rate").reshape(-1)[0])
    lcc = float(arr("log_carrying_capacity").reshape(-1)[0])
    lil = float(arr("log_initial_loss").reshape(-1)[0])

    # fp16 block
    w16 = np.zeros((128, W16COLS), np.float16)
    for nm, col in _W16_TILES:
        if nm == "negid":
            tilev = -np.eye(128, dtype=np.float32)
        elif nm.startswith("wtw"):
            base, km = nm.rsplit("_", 1)
            kk, mm = int(km[0]), int(km[1])
            if base == "wtw":
                Wf = (Ws[1] * W0[:, 1][None, :]).T
            else:
                Wf = (Ws[1] * (-2.0 * W0[:, 1] ** 2)[None, :]).T
            tilev = Wf[kk * 128 : (kk + 1) * 128, mm * 128 : (mm + 1) * 128]
        else:
            l, km = nm[2:].split("_")
            l, kk, mm = int(l), int(km[0]), int(km[1])
            Wt = Ws[l].T  # [in, out]
            tilev = Wt[kk * 128 : (kk + 1) * 128, mm * 128 : (mm + 1) * 128]
        w16[:, col : col + 128] = tilev.astype(np.float16)
    # w0ts: W0.T rows scaled by 1/(std+eps)
    w0ts = (W0.T / (in_std[:, None] + 1e-8)).astype(np.float16)  # [2, H]
    w16[0:2, _W16_W0TS : _W16_W0TS + 256] = w0ts
    # w4 halves as [128, 2]; negid handled in the tile loop below
    for kk in range(2):
        w16[:, _W16_W4 + kk] = W4[0, kk * 128 : (kk + 1) * 128].astype(np.float16)

    # fp32 block
    w32 = np.zeros((128, W32COLS), np.float32)

    def put(name, vec):
        w32[:, _W32_IDX[name]] = vec

    m0i = in_mean[0] / (in_std[0] + 1e-8)
    m1i = in_mean[1] / (in_std[1] + 1e-8)
    u = W0[:, 0] * m0i + W0[:, 1] * m1i
    beta0 = b0 - u
    put("beta0_0", beta0[0:128])
    put("beta0_1", beta0[128:256])
    for l in (1, 2, 3):
        put(f"bl{l}_0", bs[l][0:128])
        put(f"bl{l}_1", bs[l][128:256])
    r = np.exp(-lgr)
    K = 0.2 + 0.8 / (1.0 + np.exp(-lcc))
    C = 0.1 / (1.0 + np.exp(-lil))
    put("C_t", C)
    put("c1", -1.0 / (K - C))
    put("nr", -r)
    put("mc3", 2.0 * r / (K - C))
    put("sts", tgt_std[0])
    put("tmb", b4[0] * tgt_std[0] + tgt_mean[0])
    return w16, w32


def _prep_in_maps(inputs):
    w16, w32 = _pack_consts(inputs)
    x = np.asarray(inputs["inputs"], np.float32).reshape(N, 2)
    in_maps = []
    for c in range(NCORES):
        in_maps.append(
            {
                "wblk16": w16,
                "wblk32": w32,
                "x2": np.ascontiguousarray(
                    x[c * NLOC : (c + 1) * NLOC].T
                ).astype(np.float16),
            }
        )
    return in_maps


def _unshard(res_get):
    U = np.empty((N,), np.float32)
    F = np.empty((N,), np.float32)
    Ft = np.empty((N,), np.float32)
    for c in range(NCORES):
        o = res_get(c)
        U[c * NLOC : (c + 1) * NLOC] = o[0]
        F[c * NLOC : (c + 1) * NLOC] = o[1]
        Ft[c * NLOC : (c + 1) * NLOC] = o[2]
    shp = (B, S, 1)
    return U.reshape(shp), F.reshape(shp), Ft.reshape(shp)


def run(inputs, trace=False):
    nc = _get_nc()
    in_maps = _prep_in_maps(inputs)
    kw = {}
    if trace:
        kw["tmpdir"] = tempfile.mkdtemp(prefix="bassk_prof_")
    res = run_bass_kernel_spmd(
        nc, in_maps, core_ids=list(range(NCORES)), trace=trace, **kw
    )
    return _unshard(lambda c: res.results[c]["out"]), res


def kernel(**inputs):
    outs, _ = run(inputs, trace=False)
    return outs


# ---------------------------------------------------------------------------
# Dev-loop timing: persistent jitted executable (mirrors
# bass2jax.run_bass_via_pjrt's multi-core branch) so repeated executions
# reuse one compiled NEFF and can be timed back-to-back.
# ---------------------------------------------------------------------------
def _make_runner():
    if "runner" in _STATE:
        return _STATE["runner"]
    import jax
    from jax.experimental.shard_map import shard_map
    from jax.sharding import Mesh, PartitionSpec
    from concourse import bass2jax

    bass2jax.install_neuronx_cc_hook()
    nc = _get_nc()

    in_names, out_names, out_avals, zero_outs = [], [], [], []
    for alloc in nc.m.functions[0].allocations:
        if not isinstance(alloc, mybir.MemoryLocationSet):
            continue
        name = alloc.memorylocations[0].name
        if alloc.kind == "ExternalInput":
            if nc.partition_id_tensor is None or name != nc.partition_id_tensor.name:
                in_names.append(name)
        elif alloc.kind == "ExternalOutput":
            out_names.append(name)
            shape = tuple(alloc.tensor_shape)
            dtype = mybir.dt.np(alloc.dtype)
            out_avals.append(jax.core.ShapedArray(shape, dtype))
            zero_outs.append(np.zeros(shape, dtype))
    n_params = len(in_names)
    n_outs = len(out_avals)
    all_names = in_names + out_names
    if nc.partition_id_tensor is not None:
        all_names = all_names + [nc.partition_id_tensor.name]

    def _body(*args):
        operands = list(args)
        if nc.partition_id_tensor is not None:
            operands.append(bass2jax.partition_id_tensor())
        outs = bass2jax._bass_exec_p.bind(
            *operands,
            out_avals=tuple(out_avals),
            in_names=tuple(all_names),
            out_names=tuple(out_names),
            lowering_input_output_aliases=(),
            sim_require_finite=True,
            sim_require_nnan=True,
            nc=nc,
        )
        return tuple(outs)

    devices = jax.devices()[:NCORES]
    mesh = Mesh(np.asarray(devices), ("core",))
    donate = tuple(range(n_params, n_params + n_outs))
    sharded = jax.jit(
        shard_map(
            _body,
            mesh=mesh,
            in_specs=(PartitionSpec("core"),) * (n_params + n_outs),
            out_specs=(PartitionSpec("core"),) * n_outs,
            check_rep=False,
        ),
        donate_argnums=donate,
        keep_unused=True,
    )
    _STATE["runner"] = (sharded, in_names, out_names, out_avals, zero_outs)
    return _STATE["runner"]


def run_timed(inputs, iters=20):
    """Run via a persistent executable; return (outputs, per_iter_ns)."""
    import time as _time

    import jax

    sharded, in_names, out_names, out_avals, zero_outs = _make_runner()
    in_maps = _prep_in_maps(inputs)
    concat_in = [
        np.concatenate([np.asarray(in_maps[c][n]) for c in range(NCORES)], axis=0)
        for n in in_names
    ]
    dev_in = [jax.device_put(a) for a in concat_in]

    def zeros():
        return [
            np.zeros((NCORES * z.shape[0], *z.shape[1:]), z.dtype) for z in zero_outs
        ]

    # warmup (compiles on first call)
    outs = sharded(*dev_in, *zeros())
    jax.block_until_ready(outs)
    out_np = [np.asarray(o) for o in outs]

    zbufs = [zeros() for _ in range(iters)]
    t0 = _time.perf_counter()
    last = None
    for i in range(iters):
        last = sharded(*dev_in, *zbufs[i])
    jax.block_until_ready(last)
    t1 = _time.perf_counter()
    per_iter_ns = (t1 - t0) / iters * 1e9

    per_core = [
        {
            name: out_np[i].reshape(NCORES, *out_avals[i].shape)[c]
            for i, name in enumerate(out_names)
        }
        for c in range(NCORES)
    ]
    return _unshard(lambda c: per_core[c]["out"]), per_iter_ns


# revision 28
# speedup vs baseline: 1.4144x; 1.0426x over previous
"""Trainium2 Bass kernel for the CapacityNN PINN forward pass (v2).

Computes, for N = B*S collocation points x = (s, t):
  U   = MLP([s_norm, t_norm]) * tgt_std + tgt_mean
  F   = U_t  - G(U)             (G = Verhulst logistic growth term)
  F_t = U_tt - G'(U) * U_t
with U_t/U_tt computed exactly by forward-mode 2nd-order jet propagation
through the tanh MLP.

Sharding: pure data parallel over 8 NeuronCores (8192 points/core),
MLP weights + PDE scalars replicated (host-folded).

v2 layout/engine plan (from TimelineSim cost-model analysis):
  - all streams fp16 (DVE 2x tensor_tensor / 4x tensor_scalar modes)
  - [128, 1024]-wide PSUM tiles -> one Act/DVE op per stream per layer
  - elementwise jet algebra balanced across Act / DVE / Pool:
      Act : tanh, st=2*z1^2 (PSUM reads), 2 of 8 ee squares, final copies
      DVE : ee, dm=1-ee, tt=av*st, h2=dm*qt, ad0, 4 of 6 h1=dm*z1
      Pool: qt=z2-tt (PSUM read), 2 of 6 h1
  - all scalar prep + weight folding done on HOST; weights arrive as two
    pre-packed SBUF-image blocks (2 big DMAs instead of ~60 small ones)
  - tail transpose ([3,NLOC] -> [128,3*PPP]) streamed per-chunk via
    SBUF->SBUF DMAs overlapped with compute
"""

import os
import sys
import tempfile

import numpy as np

for _p in ("/opt/trn_rl_repo", "/root/.axon_site/_ro/trn_rl_repo"):
    if os.path.isdir(_p) and _p not in sys.path:
        sys.path.insert(0, _p)

import concourse.bass as bass
import concourse.bacc as bacc
import concourse.tile as tile
from concourse import mybir
from concourse.bass_utils import run_bass_kernel_spmd

AF = mybir.ActivationFunctionType
OP = mybir.AluOpType
F32 = mybir.dt.float32
F16 = mybir.dt.float16

NCORES = 8
B, S, H = 512, 128, 256
N = B * S                  # 65536 points
NLOC = N // NCORES         # 8192 points per core
CH = 1024                  # points per on-chip chunk
NCHUNK = NLOC // CH
PPP = NLOC // 128          # points per partition in the tail layout (64)
CPC = CH // 128            # tail cols per chunk (8)
SQRT2 = float(np.sqrt(2.0))

# ---- packed fp16 const block column map (must match _pack_w16) ----
# 21 [128,128] weight tiles (incl negid), then w0ts (rows 0-1), then w4 (2 cols)
_W16_TILES = []  # (name, col) in order
_c = 0
for _l in (1, 2, 3):
    for _kk in range(2):
        for _mm in range(2):
            _W16_TILES.append((f"wt{_l}_{_kk}{_mm}", _c))
            _c += 128
for _nm in ("wtw", "wtw2"):
    for _kk in range(2):
        for _mm in range(2):
            _W16_TILES.append((f"{_nm}_{_kk}{_mm}", _c))
            _c += 128
_W16_TILES.append(("negid", _c))
_c += 128
_W16_W0TS = _c          # [2, 256] at rows 0-1, cols [_c, _c+256)
_c += 256
_W16_W4 = _c            # [128, 4]: cols = [w4_k0, w4_k1, w4/sqrt2_k0, w4/sqrt2_k1]
_c += 4
W16COLS = _c

# ---- packed fp32 const block column map (must match _pack_w32) ----
# [128, W32COLS]: per-partition scalars and biases, one col each
_W32_NAMES = [
    "beta0_0", "beta0_1",
    "bl1_0", "bl1_1", "bl2_0", "bl2_1", "bl3_0", "bl3_1",
    "C_t", "c1", "nr", "mc3", "sts", "tmb",
]
W32COLS = len(_W32_NAMES)
_W32_IDX = {n: i for i, n in enumerate(_W32_NAMES)}


def _build():
    nc = bacc.Bacc(
        "TRN2",
        target_bir_lowering=False,
        debug=False,
        enable_asserts=False,
        num_devices=NCORES,
    )

    x2 = nc.dram_tensor("x2", [2, NLOC], F16, kind="ExternalInput").ap()
    wblk16 = nc.dram_tensor("wblk16", [128, W16COLS], F16, kind="ExternalInput").ap()
    wblk32 = nc.dram_tensor("wblk32", [128, W32COLS], F32, kind="ExternalInput").ap()
    out = nc.dram_tensor("out", [3, NLOC], F32, kind="ExternalOutput").ap()

    with tile.TileContext(nc) as tc:
        from contextlib import ExitStack

        with ExitStack() as ctx:
            const = ctx.enter_context(tc.tile_pool(name="const", bufs=1))
            sb = ctx.enter_context(tc.tile_pool(name="sb", bufs=1))
            ps = ctx.enter_context(tc.tile_pool(name="ps", bufs=1, space="PSUM"))

            # ---------- const loads ----------
            # split so layer-0's weights (w0ts, at the tail of the block)
            # arrive before the bulk of the hidden-layer tiles
            w16 = const.tile([128, W16COLS], F16, name="w16")
            w32 = const.tile([128, W32COLS], F32, name="w32")
            _SPLIT = _W16_TILES[-1][1]  # negid col: negid+w0ts+w4 in first DMA
            nc.sync.dma_start(out=w16[:, _SPLIT:], in_=wblk16[:, _SPLIT:])
            nc.sync.dma_start(out=w32, in_=wblk32)
            # bulk hidden-layer weights stream in behind the first x2c loads
            _wbulk = [False]

            def load_wbulk():
                if not _wbulk[0]:
                    _wbulk[0] = True
                    nc.sync.dma_start(out=w16[:, :_SPLIT], in_=wblk16[:, :_SPLIT])

            wtile = {}
            for nm, col in _W16_TILES:
                wtile[nm] = w16[:, col : col + 128]
            w0ts = w16[0:2, _W16_W0TS : _W16_W0TS + 256]  # [2, 256]
            w4c = [w16[:, _W16_W4 + kk : _W16_W4 + kk + 1] for kk in range(4)]

            def sc(name):
                i = _W32_IDX[name]
                return w32[:, i : i + 1]

            beta0 = [sc("beta0_0"), sc("beta0_1")]
            bl = {l: [sc(f"bl{l}_0"), sc(f"bl{l}_1")] for l in (1, 2, 3)}
            C_t, c1, nr, mc3, sts, tmb = (
                sc("C_t"), sc("c1"), sc("nr"), sc("mc3"), sc("sts"), sc("tmb"),
            )

            # ---------- main loop: software-pipelined chunk PAIRS ----------
            # PE/Act/DVE/Pool execute their queues in order, so matmuls and
            # elementwise consumers of the two chunks in a pair are emitted
            # interleaved: while chunk A's tanh->ee->dm->h1 chain drains,
            # the PE runs chunk B's matmuls (keeps the PE p-state ramped).
            # oc[p, c*24 + b*3 + s] = stream-s output for point c*CH + b*128 + p
            oc = sb.tile([128, 3 * PPP], F32, name="oc")
            oc2 = sb.tile([128, 3 * PPP], F32, name="oc2")

            PSB = 4   # psum [128,1024] ring (2 banks each -> all 8 banks)
            st8 = {}  # per-chunk live tiles

            def new_stream(c, tag, bufs=3):
                return sb.tile([128, CH], F16, tag=tag, bufs=bufs, name=tag)

            def l0_mm(c):
                x2c = sb.tile([2, CH], F16, tag="x2c", bufs=2)
                nc.sync.dma_start(out=x2c, in_=x2[:, c * CH : (c + 1) * CH])
                pzs = []
                for m in range(2):
                    pz = ps.tile([128, CH], F32, tag="pz", bufs=PSB, name="pz0")
                    for g in range(2):
                        nc.tensor.matmul(
                            pz[:, g * 512 : (g + 1) * 512],
                            w0ts[:, m * 128 : (m + 1) * 128],
                            x2c[:, g * 512 : (g + 1) * 512],
                            start=True,
                            stop=True,
                        )
                    pzs.append(pz)
                st8[c] = {"pz": pzs}

            def l0_cons(c):
                s = st8[c]
                Hv, H1, H2 = [None] * 2, [None] * 2, [None] * 2
                for m in range(2):
                    av = new_stream(c, f"hv{m}")
                    ee = new_stream(c, f"ee{m}", 2)
                    dm = new_stream(c, f"dm{m}", 2)
                    ad = new_stream(c, f"ad{m}", 2)
                    nc.scalar.activation(av, s["pz"][m], AF.Tanh, beta0[m])
                    nc.vector.tensor_tensor(ee, av, av, OP.mult)
                    nc.vector.tensor_scalar(dm, ee, -1.0, 1.0, OP.mult, OP.add)
                    nc.gpsimd.tensor_tensor(ad, av, dm, OP.mult)
                    Hv[m], H1[m], H2[m] = av, dm, ad
                s["Hv"], s["H1"], s["H2"] = Hv, H1, H2

            def prim_mm(c, l):
                s = st8[c]
                s["pz"] = []
                for m in range(2):
                    pz = ps.tile([128, CH], F32, tag="pz", bufs=PSB, name="pzv")
                    for g in range(2):
                        for kk in range(2):
                            nc.tensor.matmul(
                                pz[:, g * 512 : (g + 1) * 512],
                                wtile[f"wt{l}_{kk}{m}"],
                                s["Hv"][kk][:, g * 512 : (g + 1) * 512],
                                start=(kk == 0),
                                stop=(kk == 1),
                            )
                    s["pz"].append(pz)

            def tanh_emit(c, l):
                s = st8[c]
                s["nHv"] = []
                for m in range(2):
                    av = new_stream(c, f"hv{m}")
                    nc.scalar.activation(av, s["pz"][m], AF.Tanh, bl[l][m])
                    s["nHv"].append(av)

            def d1_mm(c, l):
                s = st8[c]
                s["pz1"] = []
                for m in range(2):
                    pz1 = ps.tile([128, CH], F32, tag="pz", bufs=PSB, name="pz1")
                    w1nm = f"wtw_{{}}{m}" if l == 1 else f"wt{l}_{{}}{m}"
                    for g in range(2):
                        for kk in range(2):
                            nc.tensor.matmul(
                                pz1[:, g * 512 : (g + 1) * 512],
                                wtile[w1nm.format(kk)],
                                s["H1"][kk][:, g * 512 : (g + 1) * 512],
                                start=(kk == 0),
                                stop=(kk == 1),
                            )
                    s["pz1"].append(pz1)

            def z1c_emit(c, l):
                # evacuate z1 from PSUM via one fast Act copy; frees the pz1
                # psum slot early (the old st/h1 psum reads held it ~6us).
                # The H1 stream carries a sqrt2 scale (host-folded into wtw;
                # compensated by w4/sqrt2 in the final projection) so that
                # st = z1c^2 = 2*z1_true^2 with no extra scale op.
                s = st8[c]
                s["z1c"] = []
                for m in range(2):
                    z1c = new_stream(c, f"z1c{m}", 2)
                    nc.scalar.copy(z1c, s["pz1"][m])
                    s["z1c"].append(z1c)

            def eedm_emit(c, l):
                s = st8[c]
                s["nDm"] = []
                for m in range(2):
                    ee = new_stream(c, f"ee{m}", 2)
                    dm = new_stream(c, f"dm{m}", 2)
                    if l == 1 or (l == 2 and m == 0):
                        nc.scalar.activation(ee, s["nHv"][m], AF.Square)
                    else:
                        nc.vector.tensor_tensor(ee, s["nHv"][m], s["nHv"][m], OP.mult)
                    nc.vector.tensor_scalar(dm, ee, -1.0, 1.0, OP.mult, OP.add)
                    s["nDm"].append(dm)

            def sth1_emit(c, l):
                s = st8[c]
                s["nH1"], s["st"] = [], []
                for m in range(2):
                    st = new_stream(c, f"st{m}", 2)
                    nc.vector.tensor_tensor(st, s["z1c"][m], s["z1c"][m], OP.mult)
                    s["st"].append(st)
                for m in range(2):
                    h1t = new_stream(c, f"h1{m}")
                    nc.vector.tensor_tensor(h1t, s["nDm"][m], s["z1c"][m], OP.mult)
                    s["nH1"].append(h1t)

            def tt_emit(c, l):
                s = st8[c]
                s["Tt"] = []
                for m in range(2):
                    tt = new_stream(c, f"tt{m}", 2)
                    nc.gpsimd.tensor_tensor(tt, s["nHv"][m], s["st"][m], OP.mult)
                    s["Tt"].append(tt)

            def d2_mm(c, l):
                s = st8[c]
                s["pz2"] = []
                for m in range(2):
                    pz2 = ps.tile([128, CH], F32, tag="pz", bufs=PSB, name="pz2")
                    w2nm = f"wtw2_{{}}{m}" if l == 1 else f"wt{l}_{{}}{m}"
                    for g in range(2):
                        for kk in range(2):
                            nc.tensor.matmul(
                                pz2[:, g * 512 : (g + 1) * 512],
                                wtile[w2nm.format(kk)],
                                s["H2"][kk][:, g * 512 : (g + 1) * 512],
                                start=(kk == 0),
                                stop=False,
                            )
                    s["pz2"].append(pz2)

            def negid_mm(c, l):
                # qt = z2 - tt: accumulate -I @ tt into the z2 psum group
                s = st8[c]
                for m in range(2):
                    for g in range(2):
                        nc.tensor.matmul(
                            s["pz2"][m][:, g * 512 : (g + 1) * 512],
                            wtile["negid"],
                            s["Tt"][m][:, g * 512 : (g + 1) * 512],
                            start=False,
                            stop=True,
                        )

            def h2_emit(c, l):
                s = st8[c]
                s["nH2"] = []
                for m in range(2):
                    h2t = new_stream(c, f"h2{m}")
                    nc.vector.tensor_tensor(h2t, s["nDm"][m], s["pz2"][m], OP.mult)
                    s["nH2"].append(h2t)

            def layer_rotate(c):
                s = st8[c]
                s["Hv"], s["H1"], s["H2"] = s["nHv"], s["nH1"], s["nH2"]

            def final_mm(c):
                s = st8[c]
                pblk = ps.tile([128, 3 * CPC], F32, tag="pz", bufs=PSB, name="pblk")
                for b in range(CPC):
                    for s_idx, stream in enumerate((s["Hv"], s["H1"], s["H2"])):
                        for kk in range(2):
                            nc.tensor.matmul(
                                pblk[:, b * 3 + s_idx : b * 3 + s_idx + 1],
                                stream[kk][:, b * 128 : (b + 1) * 128],
                                w4c[(2 if s_idx == 1 else 0) + kk],
                                start=(kk == 0),
                                stop=(kk == 1),
                            )
                s["pblk"] = pblk

            def final_copy(c):
                nc.scalar.copy(
                    oc[:, c * 3 * CPC : (c + 1) * 3 * CPC], st8[c]["pblk"]
                )

            def pair_pde(c0):
                # PDE algebra for this pair's [128, 48] slice of oc, on Pool
                # (Pool has slack; keeps the end-of-kernel tail tiny), then
                # the output DMAs for these two chunks.
                lo = c0 * 3 * CPC
                n = 2 * 3 * CPC
                osl = slice(lo, lo + n)
                ocp = oc[:, osl]
                ocq = oc2[:, osl]
                yv = ocp[:, 0:n:3]
                yt = ocp[:, 1:n:3]
                ytt = ocp[:, 2:n:3]
                U = ocq[:, 0:n:3]
                Fo = ocq[:, 1:n:3]
                Ft = ocq[:, 2:n:3]
                k = 2 * CPC

                def tl(name):
                    return sb.tile([128, k], F32, tag=name, bufs=2, name=name)

                ut, utt, vv, v2, w1, q1, t1 = (
                    tl("ut"), tl("utt"), tl("vv"), tl("v2"),
                    tl("w1"), tl("q1"), tl("t1"),
                )
                ve = nc.vector
                ve.tensor_scalar(U, yv, sts, tmb, OP.mult, OP.add)
                ve.tensor_scalar(ut, yt, sts, None, OP.mult)
                ve.tensor_scalar(utt, ytt, sts, None, OP.mult)
                ve.tensor_scalar(vv, U, C_t, None, OP.subtract)
                nc.gpsimd.tensor_tensor(v2, vv, vv, OP.mult)
                ve.scalar_tensor_tensor(w1, v2, c1, vv, OP.mult, OP.add)
                ve.scalar_tensor_tensor(Fo, w1, nr, ut, OP.mult, OP.add)
                nc.gpsimd.tensor_tensor(q1, vv, ut, OP.mult)
                ve.scalar_tensor_tensor(t1, ut, nr, utt, OP.mult, OP.add)
                ve.scalar_tensor_tensor(Ft, q1, mc3, t1, OP.mult, OP.add)
                # out[s, c*CH + b*128 + p] = oc2[p, c*3*CPC + b*3 + s]
                for s_idx in range(3):
                    nc.sync.dma_start(
                        out=bass.AP(
                            out.tensor,
                            s_idx * NLOC + c0 * CH,
                            [[1, 128], [CH, 2], [128, CPC]],
                        ),
                        in_=bass.AP(
                            oc2.tensor,
                            oc2.offset + lo + s_idx,
                            [list(oc2.ap[0]), [3 * CPC, 2], [3, CPC]],
                        ),
                    )

            for c0 in range(0, NCHUNK, 2):
                pair = (c0, c0 + 1)
                for c in pair:
                    l0_mm(c)
                load_wbulk()
                for c in pair:
                    l0_cons(c)
                for l in (1, 2, 3):
                    for c in pair:
                        prim_mm(c, l)
                    for c in pair:
                        tanh_emit(c, l)
                    for c in pair:
                        d1_mm(c, l)
                    for c in pair:
                        z1c_emit(c, l)
                    for c in pair:
                        eedm_emit(c, l)
                    for c in pair:
                        sth1_emit(c, l)
                        tt_emit(c, l)
                    last = l == 3
                    for c in pair:
                        d2_mm(c, l)
                        negid_mm(c, l)
                        h2_emit(c, l)
                        layer_rotate(c)
                        if last:
                            final_mm(c)
                    if last:
                        for c in pair:
                            final_copy(c)
                if c0 > 0:
                    pair_pde(c0 - 2)
            pair_pde(NCHUNK - 2)

    nc.compile()
    return nc


_STATE = {}


def _get_nc():
    if "nc" not in _STATE:
        _STATE["nc"] = _build()
    return _STATE["nc"]


def _pack_consts(inputs):
    f = np.float32

    def arr(k):
        return np.ascontiguousarray(np.asarray(inputs[k], f))

    W0, b0 = arr("W0"), arr("b0")
    Ws = {1: arr("W1"), 2: arr("W2"), 3: arr("W3")}
    bs = {1: arr("b1"), 2: arr("b2"), 3: arr("b3")}
    W4, b4 = arr("W4").reshape(1, H), arr("b4").reshape(1)
    in_mean, in_std = arr("in_mean"), arr("in_std")
    tgt_mean, tgt_std = arr("tgt_mean"), arr("tgt_std")
    lgr = float(arr("log_growth_rate").reshape(-1)[0])
    lcc = float(arr("log_carrying_capacity").reshape(-1)[0])
    lil = float(arr("log_initial_loss").reshape(-1)[0])

    # fp16 block
    w16 = np.zeros((128, W16COLS), np.float16)
    for nm, col in _W16_TILES:
        if nm == "negid":
            tilev = -np.eye(128, dtype=np.float32)
        elif nm.startswith("wtw"):
            base, km = nm.rsplit("_", 1)
            kk, mm = int(km[0]), int(km[1])
            if base == "wtw":
                # sqrt2-scaled H1 stream: st = (sqrt2*z1)^2 = 2*z1^2 for free
                Wf = (Ws[1] * (SQRT2 * W0[:, 1])[None, :]).T
            else:
                Wf = (Ws[1] * (-2.0 * W0[:, 1] ** 2)[None, :]).T
            tilev = Wf[kk * 128 : (kk + 1) * 128, mm * 128 : (mm + 1) * 128]
        else:
            l, km = nm[2:].split("_")
            l, kk, mm = int(l), int(km[0]), int(km[1])
            Wt = Ws[l].T  # [in, out]
            tilev = Wt[kk * 128 : (kk + 1) * 128, mm * 128 : (mm + 1) * 128]
        w16[:, col : col + 128] = tilev.astype(np.float16)
    # w0ts: W0.T rows scaled by 1/(std+eps)
    w0ts = (W0.T / (in_std[:, None] + 1e-8)).astype(np.float16)  # [2, H]
    w16[0:2, _W16_W0TS : _W16_W0TS + 256] = w0ts
    # w4 halves; cols 2-3 carry 1/sqrt2 to undo the H1 stream's sqrt2 scale
    for kk in range(2):
        w4h = W4[0, kk * 128 : (kk + 1) * 128]
        w16[:, _W16_W4 + kk] = w4h.astype(np.float16)
        w16[:, _W16_W4 + 2 + kk] = (w4h / SQRT2).astype(np.float16)

    # fp32 block
    w32 = np.zeros((128, W32COLS), np.float32)

    def put(name, vec):
        w32[:, _W32_IDX[name]] = vec

    m0i = in_mean[0] / (in_std[0] + 1e-8)
    m1i = in_mean[1] / (in_std[1] + 1e-8)
    u = W0[:, 0] * m0i + W0[:, 1] * m1i
    beta0 = b0 - u
    put("beta0_0", beta0[0:128])
    put("beta0_1", beta0[128:256])
    for l in (1, 2, 3):
        put(f"bl{l}_0", bs[l][0:128])
        put(f"bl{l}_1", bs[l][128:256])
    r = np.exp(-lgr)
    K = 0.2 + 0.8 / (1.0 + np.exp(-lcc))
    C = 0.1 / (1.0 + np.exp(-lil))
    put("C_t", C)
    put("c1", -1.0 / (K - C))
    put("nr", -r)
    put("mc3", 2.0 * r / (K - C))
    put("sts", tgt_std[0])
    put("tmb", b4[0] * tgt_std[0] + tgt_mean[0])
    return w16, w32


def _prep_in_maps(inputs):
    w16, w32 = _pack_consts(inputs)
    x = np.asarray(inputs["inputs"], np.float32).reshape(N, 2)
    in_maps = []
    for c in range(NCORES):
        in_maps.append(
            {
                "wblk16": w16,
                "wblk32": w32,
                "x2": np.ascontiguousarray(
                    x[c * NLOC : (c + 1) * NLOC].T
                ).astype(np.float16),
            }
        )
    return in_maps


def _unshard(res_get):
    U = np.empty((N,), np.float32)
    F = np.empty((N,), np.float32)
    Ft = np.empty((N,), np.float32)
    for c in range(NCORES):
        o = res_get(c)
        U[c * NLOC : (c + 1) * NLOC] = o[0]
        F[c * NLOC : (c + 1) * NLOC] = o[1]
        Ft[c * NLOC : (c + 1) * NLOC] = o[2]
    shp = (B, S, 1)
    return U.reshape(shp), F.reshape(shp), Ft.reshape(shp)


def run(inputs, trace=False):
    nc = _get_nc()
    in_maps = _prep_in_maps(inputs)
    kw = {}
    if trace:
        kw["tmpdir"] = tempfile.mkdtemp(prefix="bassk_prof_")
    res = run_bass_kernel_spmd(
        nc, in_maps, core_ids=list(range(NCORES)), trace=trace, **kw
    )
    return _unshard(lambda c: res.results[c]["out"]), res


def kernel(**inputs):
    outs, _ = run(inputs, trace=False)
    return outs


# ---------------------------------------------------------------------------
# Dev-loop timing: persistent jitted executable (mirrors
# bass2jax.run_bass_via_pjrt's multi-core branch) so repeated executions
# reuse one compiled NEFF and can be timed back-to-back.
# ---------------------------------------------------------------------------
def _make_runner():
    if "runner" in _STATE:
        return _STATE["runner"]
    import jax
    from jax.experimental.shard_map import shard_map
    from jax.sharding import Mesh, PartitionSpec
    from concourse import bass2jax

    bass2jax.install_neuronx_cc_hook()
    nc = _get_nc()

    in_names, out_names, out_avals, zero_outs = [], [], [], []
    for alloc in nc.m.functions[0].allocations:
        if not isinstance(alloc, mybir.MemoryLocationSet):
            continue
        name = alloc.memorylocations[0].name
        if alloc.kind == "ExternalInput":
            if nc.partition_id_tensor is None or name != nc.partition_id_tensor.name:
                in_names.append(name)
        elif alloc.kind == "ExternalOutput":
            out_names.append(name)
            shape = tuple(alloc.tensor_shape)
            dtype = mybir.dt.np(alloc.dtype)
            out_avals.append(jax.core.ShapedArray(shape, dtype))
            zero_outs.append(np.zeros(shape, dtype))
    n_params = len(in_names)
    n_outs = len(out_avals)
    all_names = in_names + out_names
    if nc.partition_id_tensor is not None:
        all_names = all_names + [nc.partition_id_tensor.name]

    def _body(*args):
        operands = list(args)
        if nc.partition_id_tensor is not None:
            operands.append(bass2jax.partition_id_tensor())
        outs = bass2jax._bass_exec_p.bind(
            *operands,
            out_avals=tuple(out_avals),
            in_names=tuple(all_names),
            out_names=tuple(out_names),
            lowering_input_output_aliases=(),
            sim_require_finite=True,
            sim_require_nnan=True,
            nc=nc,
        )
        return tuple(outs)

    devices = jax.devices()[:NCORES]
    mesh = Mesh(np.asarray(devices), ("core",))
    donate = tuple(range(n_params, n_params + n_outs))
    sharded = jax.jit(
        shard_map(
            _body,
            mesh=mesh,
            in_specs=(PartitionSpec("core"),) * (n_params + n_outs),
            out_specs=(PartitionSpec("core"),) * n_outs,
            check_rep=False,
        ),
        donate_argnums=donate,
        keep_unused=True,
    )
    _STATE["runner"] = (sharded, in_names, out_names, out_avals, zero_outs)
    return _STATE["runner"]


def run_timed(inputs, iters=20):
    """Run via a persistent executable; return (outputs, per_iter_ns)."""
    import time as _time

    import jax

    sharded, in_names, out_names, out_avals, zero_outs = _make_runner()
    in_maps = _prep_in_maps(inputs)
    concat_in = [
        np.concatenate([np.asarray(in_maps[c][n]) for c in range(NCORES)], axis=0)
        for n in in_names
    ]
    dev_in = [jax.device_put(a) for a in concat_in]

    def zeros():
        return [
            np.zeros((NCORES * z.shape[0], *z.shape[1:]), z.dtype) for z in zero_outs
        ]

    # warmup (compiles on first call)
    outs = sharded(*dev_in, *zeros())
    jax.block_until_ready(outs)
    out_np = [np.asarray(o) for o in outs]

    zbufs = [zeros() for _ in range(iters)]
    t0 = _time.perf_counter()
    last = None
    for i in range(iters):
        last = sharded(*dev_in, *zbufs[i])
    jax.block_until_ready(last)
    t1 = _time.perf_counter()
    per_iter_ns = (t1 - t0) / iters * 1e9

    per_core = [
        {
            name: out_np[i].reshape(NCORES, *out_avals[i].shape)[c]
            for i, name in enumerate(out_names)
        }
        for c in range(NCORES)
    ]
    return _unshard(lambda c: per_core[c]["out"]), per_iter_ns


# revision 33
# speedup vs baseline: 1.4888x; 1.0526x over previous
"""Trainium2 Bass kernel for the CapacityNN PINN forward pass (v2).

Computes, for N = B*S collocation points x = (s, t):
  U   = MLP([s_norm, t_norm]) * tgt_std + tgt_mean
  F   = U_t  - G(U)             (G = Verhulst logistic growth term)
  F_t = U_tt - G'(U) * U_t
with U_t/U_tt computed exactly by forward-mode 2nd-order jet propagation
through the tanh MLP.

Sharding: pure data parallel over 8 NeuronCores (8192 points/core),
MLP weights + PDE scalars replicated (host-folded).

v2 layout/engine plan (from TimelineSim cost-model analysis):
  - all streams fp16 (DVE 2x tensor_tensor / 4x tensor_scalar modes)
  - [128, 1024]-wide PSUM tiles -> one Act/DVE op per stream per layer
  - elementwise jet algebra balanced across Act / DVE / Pool:
      Act : tanh, st=2*z1^2 (PSUM reads), 2 of 8 ee squares, final copies
      DVE : ee, dm=1-ee, tt=av*st, h2=dm*qt, ad0, 4 of 6 h1=dm*z1
      Pool: qt=z2-tt (PSUM read), 2 of 6 h1
  - all scalar prep + weight folding done on HOST; weights arrive as two
    pre-packed SBUF-image blocks (2 big DMAs instead of ~60 small ones)
  - tail transpose ([3,NLOC] -> [128,3*PPP]) streamed per-chunk via
    SBUF->SBUF DMAs overlapped with compute
"""

import os
import sys
import tempfile

import numpy as np

for _p in ("/opt/trn_rl_repo", "/root/.axon_site/_ro/trn_rl_repo"):
    if os.path.isdir(_p) and _p not in sys.path:
        sys.path.insert(0, _p)

import concourse.bass as bass
import concourse.bacc as bacc
import concourse.tile as tile
from concourse import mybir
from concourse.bass_utils import run_bass_kernel_spmd

AF = mybir.ActivationFunctionType
OP = mybir.AluOpType
F32 = mybir.dt.float32
F16 = mybir.dt.float16

NCORES = 8
B, S, H = 512, 128, 256
N = B * S                  # 65536 points
NLOC = N // NCORES         # 8192 points per core
CH = 1024                  # points per on-chip chunk
NCHUNK = NLOC // CH
PPP = NLOC // 128          # points per partition in the tail layout (64)
CPC = CH // 128            # tail cols per chunk (8)
SQRT2 = float(np.sqrt(2.0))

# ---- packed fp16 const block column map (must match _pack_w16) ----
# 21 [128,128] weight tiles (incl negid), then w0ts (rows 0-1), then w4 (2 cols)
_W16_TILES = []  # (name, col) in order
_c = 0
for _l in (1, 2, 3):
    for _kk in range(2):
        for _mm in range(2):
            _W16_TILES.append((f"wt{_l}_{_kk}{_mm}", _c))
            _c += 128
for _nm in ("wtw", "wtw2"):
    for _kk in range(2):
        for _mm in range(2):
            _W16_TILES.append((f"{_nm}_{_kk}{_mm}", _c))
            _c += 128
_W16_TILES.append(("negid", _c))
_c += 128
_W16_W0TS = _c          # [2, 256] at rows 0-1, cols [_c, _c+256)
_c += 256
_W16_W4 = _c            # [128, 4]: cols = [w4_k0, w4_k1, w4/sqrt2_k0, w4/sqrt2_k1]
_c += 4
W16COLS = _c

# ---- packed fp32 const block column map (must match _pack_w32) ----
# [128, W32COLS]: per-partition scalars and biases, one col each
_W32_NAMES = [
    "beta0_0", "beta0_1",
    "bl1_0", "bl1_1", "bl2_0", "bl2_1", "bl3_0", "bl3_1",
    "C_t", "c1", "nr", "mc3", "sts", "tmb",
]
W32COLS = len(_W32_NAMES)
_W32_IDX = {n: i for i, n in enumerate(_W32_NAMES)}


def _build():
    nc = bacc.Bacc(
        "TRN2",
        target_bir_lowering=False,
        debug=False,
        enable_asserts=False,
        num_devices=NCORES,
    )

    x2 = nc.dram_tensor("x2", [2, NLOC], F16, kind="ExternalInput").ap()
    wblk16 = nc.dram_tensor("wblk16", [128, W16COLS], F16, kind="ExternalInput").ap()
    wblk32 = nc.dram_tensor("wblk32", [128, W32COLS], F32, kind="ExternalInput").ap()
    out = nc.dram_tensor("out", [3, NLOC], F32, kind="ExternalOutput").ap()

    with tile.TileContext(nc) as tc:
        from contextlib import ExitStack

        with ExitStack() as ctx:
            const = ctx.enter_context(tc.tile_pool(name="const", bufs=1))
            sb = ctx.enter_context(tc.tile_pool(name="sb", bufs=1))
            ps = ctx.enter_context(tc.tile_pool(name="ps", bufs=1, space="PSUM"))

            # ---------- const loads ----------
            # split so layer-0's weights (w0ts, at the tail of the block)
            # arrive before the bulk of the hidden-layer tiles
            w16 = const.tile([128, W16COLS], F16, name="w16")
            w32 = const.tile([128, W32COLS], F32, name="w32")
            _SPLIT = _W16_TILES[-1][1]  # negid col: negid+w0ts+w4 in first DMA
            nc.sync.dma_start(out=w16[:, _SPLIT:], in_=wblk16[:, _SPLIT:])
            nc.sync.dma_start(out=w32, in_=wblk32)
            # bulk hidden-layer weights stream in behind the first x2c loads
            _wbulk = [False]

            def load_wbulk():
                if not _wbulk[0]:
                    _wbulk[0] = True
                    nc.sync.dma_start(out=w16[:, :_SPLIT], in_=wblk16[:, :_SPLIT])

            wtile = {}
            for nm, col in _W16_TILES:
                wtile[nm] = w16[:, col : col + 128]
            w0ts = w16[0:2, _W16_W0TS : _W16_W0TS + 256]  # [2, 256]
            w4c = [w16[:, _W16_W4 + kk : _W16_W4 + kk + 1] for kk in range(4)]

            def sc(name):
                i = _W32_IDX[name]
                return w32[:, i : i + 1]

            beta0 = [sc("beta0_0"), sc("beta0_1")]
            bl = {l: [sc(f"bl{l}_0"), sc(f"bl{l}_1")] for l in (1, 2, 3)}
            C_t, c1, nr, mc3, sts, tmb = (
                sc("C_t"), sc("c1"), sc("nr"), sc("mc3"), sc("sts"), sc("tmb"),
            )

            # ---------- main loop: software-pipelined chunk PAIRS ----------
            # PE/Act/DVE/Pool execute their queues in order, so matmuls and
            # elementwise consumers of the two chunks in a pair are emitted
            # interleaved: while chunk A's tanh->ee->dm->h1 chain drains,
            # the PE runs chunk B's matmuls (keeps the PE p-state ramped).
            # oc[p, c*24 + b*3 + s] = stream-s output for point c*CH + b*128 + p
            oc = sb.tile([128, 3 * PPP], F32, name="oc")
            oc2 = sb.tile([128, 3 * PPP], F32, name="oc2")

            PSB = 4   # psum [128,1024] ring (2 banks each -> all 8 banks)
            st8 = {}  # per-chunk live tiles

            def new_stream(c, tag, bufs=4):
                return sb.tile([128, CH], F16, tag=tag, bufs=bufs, name=tag)

            def l0_mm(c):
                x2c = sb.tile([2, CH], F16, tag="x2c", bufs=3)
                nc.sync.dma_start(out=x2c, in_=x2[:, c * CH : (c + 1) * CH])
                pzs = []
                for m in range(2):
                    pz = ps.tile([128, CH], F32, tag="pz", bufs=PSB, name="pz0")
                    for g in range(2):
                        nc.tensor.matmul(
                            pz[:, g * 512 : (g + 1) * 512],
                            w0ts[:, m * 128 : (m + 1) * 128],
                            x2c[:, g * 512 : (g + 1) * 512],
                            start=True,
                            stop=True,
                        )
                    pzs.append(pz)
                st8[c] = {"pz": pzs}

            def l0_cons(c):
                s = st8[c]
                Hv, H1, H2 = [None] * 2, [None] * 2, [None] * 2
                for m in range(2):
                    av = new_stream(c, f"hv{m}")
                    ee = new_stream(c, f"ee{m}", 3)
                    dm = new_stream(c, f"dm{m}", 3)
                    ad = new_stream(c, f"ad{m}", 3)
                    nc.scalar.activation(av, s["pz"][m], AF.Tanh, beta0[m])
                    nc.vector.tensor_tensor(ee, av, av, OP.mult)
                    nc.vector.tensor_scalar(dm, ee, -1.0, 1.0, OP.mult, OP.add)
                    nc.gpsimd.tensor_tensor(ad, av, dm, OP.mult)
                    Hv[m], H1[m], H2[m] = av, dm, ad
                s["Hv"], s["H1"], s["H2"] = Hv, H1, H2

            def prim_mm(c, l):
                s = st8[c]
                s["pz"] = []
                for m in range(2):
                    pz = ps.tile([128, CH], F32, tag="pz", bufs=PSB, name="pzv")
                    for g in range(2):
                        for kk in range(2):
                            nc.tensor.matmul(
                                pz[:, g * 512 : (g + 1) * 512],
                                wtile[f"wt{l}_{kk}{m}"],
                                s["Hv"][kk][:, g * 512 : (g + 1) * 512],
                                start=(kk == 0),
                                stop=(kk == 1),
                            )
                    s["pz"].append(pz)

            def tanh_emit(c, l):
                s = st8[c]
                s["nHv"] = []
                for m in range(2):
                    av = new_stream(c, f"hv{m}")
                    nc.scalar.activation(av, s["pz"][m], AF.Tanh, bl[l][m])
                    s["nHv"].append(av)

            def d1_mm(c, l):
                s = st8[c]
                s["pz1"] = []
                for m in range(2):
                    pz1 = ps.tile([128, CH], F32, tag="pz", bufs=PSB, name="pz1")
                    w1nm = f"wtw_{{}}{m}" if l == 1 else f"wt{l}_{{}}{m}"
                    for g in range(2):
                        for kk in range(2):
                            nc.tensor.matmul(
                                pz1[:, g * 512 : (g + 1) * 512],
                                wtile[w1nm.format(kk)],
                                s["H1"][kk][:, g * 512 : (g + 1) * 512],
                                start=(kk == 0),
                                stop=(kk == 1),
                            )
                    s["pz1"].append(pz1)

            def z1c_emit(c, l):
                # evacuate z1 from PSUM via one fast Act copy; frees the pz1
                # psum slot early (the old st/h1 psum reads held it ~6us).
                # The H1 stream carries a sqrt2 scale (host-folded into wtw;
                # compensated by w4/sqrt2 in the final projection) so that
                # st = z1c^2 = 2*z1_true^2 with no extra scale op.
                s = st8[c]
                s["z1c"] = []
                for m in range(2):
                    z1c = new_stream(c, f"z1c{m}", 3)
                    nc.scalar.copy(z1c, s["pz1"][m])
                    s["z1c"].append(z1c)

            def eedm_emit(c, l):
                s = st8[c]
                s["nDm"] = []
                for m in range(2):
                    ee = new_stream(c, f"ee{m}", 3)
                    dm = new_stream(c, f"dm{m}", 3)
                    if l == 1 or (l == 2 and m == 0):
                        nc.scalar.activation(ee, s["nHv"][m], AF.Square)
                    else:
                        nc.vector.tensor_tensor(ee, s["nHv"][m], s["nHv"][m], OP.mult)
                    nc.vector.tensor_scalar(dm, ee, -1.0, 1.0, OP.mult, OP.add)
                    s["nDm"].append(dm)

            def sth1_emit(c, l):
                s = st8[c]
                s["nH1"], s["st"] = [], []
                for m in range(2):
                    st = new_stream(c, f"st{m}", 3)
                    nc.vector.tensor_tensor(st, s["z1c"][m], s["z1c"][m], OP.mult)
                    s["st"].append(st)
                for m in range(2):
                    h1t = new_stream(c, f"h1{m}")
                    nc.vector.tensor_tensor(h1t, s["nDm"][m], s["z1c"][m], OP.mult)
                    s["nH1"].append(h1t)

            def tt_emit(c, l):
                s = st8[c]
                s["Tt"] = []
                for m in range(2):
                    tt = new_stream(c, f"tt{m}", 3)
                    nc.gpsimd.tensor_tensor(tt, s["nHv"][m], s["st"][m], OP.mult)
                    s["Tt"].append(tt)

            def d2_mm(c, l):
                s = st8[c]
                s["pz2"] = []
                for m in range(2):
                    pz2 = ps.tile([128, CH], F32, tag="pz", bufs=PSB, name="pz2")
                    w2nm = f"wtw2_{{}}{m}" if l == 1 else f"wt{l}_{{}}{m}"
                    for g in range(2):
                        for kk in range(2):
                            nc.tensor.matmul(
                                pz2[:, g * 512 : (g + 1) * 512],
                                wtile[w2nm.format(kk)],
                                s["H2"][kk][:, g * 512 : (g + 1) * 512],
                                start=(kk == 0),
                                stop=False,
                            )
                    s["pz2"].append(pz2)

            def negid_mm(c, l):
                # qt = z2 - tt: accumulate -I @ tt into the z2 psum group
                s = st8[c]
                for m in range(2):
                    for g in range(2):
                        nc.tensor.matmul(
                            s["pz2"][m][:, g * 512 : (g + 1) * 512],
                            wtile["negid"],
                            s["Tt"][m][:, g * 512 : (g + 1) * 512],
                            start=False,
                            stop=True,
                        )

            def h2_emit(c, l):
                s = st8[c]
                s["nH2"] = []
                for m in range(2):
                    h2t = new_stream(c, f"h2{m}")
                    nc.vector.tensor_tensor(h2t, s["nDm"][m], s["pz2"][m], OP.mult)
                    s["nH2"].append(h2t)

            def layer_rotate(c):
                s = st8[c]
                s["Hv"], s["H1"], s["H2"] = s["nHv"], s["nH1"], s["nH2"]

            def final_mm(c):
                s = st8[c]
                pblk = ps.tile([128, 3 * CPC], F32, tag="pz", bufs=PSB, name="pblk")
                for b in range(CPC):
                    for s_idx, stream in enumerate((s["Hv"], s["H1"], s["H2"])):
                        for kk in range(2):
                            nc.tensor.matmul(
                                pblk[:, b * 3 + s_idx : b * 3 + s_idx + 1],
                                stream[kk][:, b * 128 : (b + 1) * 128],
                                w4c[(2 if s_idx == 1 else 0) + kk],
                                start=(kk == 0),
                                stop=(kk == 1),
                            )
                s["pblk"] = pblk

            def final_copy(c):
                nc.scalar.copy(
                    oc[:, c * 3 * CPC : (c + 1) * 3 * CPC], st8[c]["pblk"]
                )

            def pair_pde(c0, ng=2):
                # PDE algebra for this group's slice of oc (DVE + Pool for
                # the tensor_tensor ops), then the output DMAs.
                lo = c0 * 3 * CPC
                n = ng * 3 * CPC
                osl = slice(lo, lo + n)
                ocp = oc[:, osl]
                ocq = oc2[:, osl]
                yv = ocp[:, 0:n:3]
                yt = ocp[:, 1:n:3]
                ytt = ocp[:, 2:n:3]
                U = ocq[:, 0:n:3]
                Fo = ocq[:, 1:n:3]
                Ft = ocq[:, 2:n:3]
                k = ng * CPC

                def tl(name):
                    return sb.tile([128, k], F32, tag=name, bufs=2, name=name)

                ut, utt, vv, v2, w1, q1, t1 = (
                    tl("ut"), tl("utt"), tl("vv"), tl("v2"),
                    tl("w1"), tl("q1"), tl("t1"),
                )
                ve = nc.vector
                ve.tensor_scalar(U, yv, sts, tmb, OP.mult, OP.add)
                ve.tensor_scalar(ut, yt, sts, None, OP.mult)
                ve.tensor_scalar(utt, ytt, sts, None, OP.mult)
                ve.tensor_scalar(vv, U, C_t, None, OP.subtract)
                nc.gpsimd.tensor_tensor(v2, vv, vv, OP.mult)
                ve.scalar_tensor_tensor(w1, v2, c1, vv, OP.mult, OP.add)
                ve.scalar_tensor_tensor(Fo, w1, nr, ut, OP.mult, OP.add)
                nc.gpsimd.tensor_tensor(q1, vv, ut, OP.mult)
                ve.scalar_tensor_tensor(t1, ut, nr, utt, OP.mult, OP.add)
                ve.scalar_tensor_tensor(Ft, q1, mc3, t1, OP.mult, OP.add)
                # out[s, c*CH + b*128 + p] = oc2[p, c*3*CPC + b*3 + s]
                for s_idx in range(3):
                    nc.sync.dma_start(
                        out=bass.AP(
                            out.tensor,
                            s_idx * NLOC + c0 * CH,
                            [[1, 128], [CH, ng], [128, CPC]],
                        ),
                        in_=bass.AP(
                            oc2.tensor,
                            oc2.offset + lo + s_idx,
                            [list(oc2.ap[0]), [3 * CPC, ng], [3, CPC]],
                        ),
                    )

            groups = [(0, 1, 2), (3, 4, 5), (6, 7)]
            prev = None
            for grp in groups:
                for c in grp:
                    l0_mm(c)
                load_wbulk()
                for c in grp:
                    l0_cons(c)
                for l in (1, 2, 3):
                    for c in grp:
                        prim_mm(c, l)
                    for c in grp:
                        tanh_emit(c, l)
                    for c in grp:
                        d1_mm(c, l)
                    for c in grp:
                        z1c_emit(c, l)
                    for c in grp:
                        eedm_emit(c, l)
                    for c in grp:
                        sth1_emit(c, l)
                        tt_emit(c, l)
                    last = l == 3
                    for c in grp:
                        d2_mm(c, l)
                        negid_mm(c, l)
                        h2_emit(c, l)
                        layer_rotate(c)
                        if last:
                            final_mm(c)
                    if last:
                        for c in grp:
                            final_copy(c)
                if prev is not None:
                    pair_pde(prev[0], len(prev))
                prev = grp
            pair_pde(prev[0], len(prev))

    nc.compile()
    return nc


_STATE = {}


def _get_nc():
    if "nc" not in _STATE:
        _STATE["nc"] = _build()
    return _STATE["nc"]


def _pack_consts(inputs):
    f = np.float32

    def arr(k):
        return np.ascontiguousarray(np.asarray(inputs[k], f))

    W0, b0 = arr("W0"), arr("b0")
    Ws = {1: arr("W1"), 2: arr("W2"), 3: arr("W3")}
    bs = {1: arr("b1"), 2: arr("b2"), 3: arr("b3")}
    W4, b4 = arr("W4").reshape(1, H), arr("b4").reshape(1)
    in_mean, in_std = arr("in_mean"), arr("in_std")
    tgt_mean, tgt_std = arr("tgt_mean"), arr("tgt_std")
    lgr = float(arr("log_growth_rate").reshape(-1)[0])
    lcc = float(arr("log_carrying_capacity").reshape(-1)[0])
    lil = float(arr("log_initial_loss").reshape(-1)[0])

    # fp16 block
    w16 = np.zeros((128, W16COLS), np.float16)
    for nm, col in _W16_TILES:
        if nm == "negid":
            tilev = -np.eye(128, dtype=np.float32)
        elif nm.startswith("wtw"):
            base, km = nm.rsplit("_", 1)
            kk, mm = int(km[0]), int(km[1])
            if base == "wtw":
                # sqrt2-scaled H1 stream: st = (sqrt2*z1)^2 = 2*z1^2 for free
                Wf = (Ws[1] * (SQRT2 * W0[:, 1])[None, :]).T
            else:
                Wf = (Ws[1] * (-2.0 * W0[:, 1] ** 2)[None, :]).T
            tilev = Wf[kk * 128 : (kk + 1) * 128, mm * 128 : (mm + 1) * 128]
        else:
            l, km = nm[2:].split("_")
            l, kk, mm = int(l), int(km[0]), int(km[1])
            Wt = Ws[l].T  # [in, out]
            tilev = Wt[kk * 128 : (kk + 1) * 128, mm * 128 : (mm + 1) * 128]
        w16[:, col : col + 128] = tilev.astype(np.float16)
    # w0ts: W0.T rows scaled by 1/(std+eps)
    w0ts = (W0.T / (in_std[:, None] + 1e-8)).astype(np.float16)  # [2, H]
    w16[0:2, _W16_W0TS : _W16_W0TS + 256] = w0ts
    # w4 halves; cols 2-3 carry 1/sqrt2 to undo the H1 stream's sqrt2 scale
    for kk in range(2):
        w4h = W4[0, kk * 128 : (kk + 1) * 128]
        w16[:, _W16_W4 + kk] = w4h.astype(np.float16)
        w16[:, _W16_W4 + 2 + kk] = (w4h / SQRT2).astype(np.float16)

    # fp32 block
    w32 = np.zeros((128, W32COLS), np.float32)

    def put(name, vec):
        w32[:, _W32_IDX[name]] = vec

    m0i = in_mean[0] / (in_std[0] + 1e-8)
    m1i = in_mean[1] / (in_std[1] + 1e-8)
    u = W0[:, 0] * m0i + W0[:, 1] * m1i
    beta0 = b0 - u
    put("beta0_0", beta0[0:128])
    put("beta0_1", beta0[128:256])
    for l in (1, 2, 3):
        put(f"bl{l}_0", bs[l][0:128])
        put(f"bl{l}_1", bs[l][128:256])
    r = np.exp(-lgr)
    K = 0.2 + 0.8 / (1.0 + np.exp(-lcc))
    C = 0.1 / (1.0 + np.exp(-lil))
    put("C_t", C)
    put("c1", -1.0 / (K - C))
    put("nr", -r)
    put("mc3", 2.0 * r / (K - C))
    put("sts", tgt_std[0])
    put("tmb", b4[0] * tgt_std[0] + tgt_mean[0])
    return w16, w32


def _prep_in_maps(inputs):
    w16, w32 = _pack_consts(inputs)
    x = np.asarray(inputs["inputs"], np.float32).reshape(N, 2)
    in_maps = []
    for c in range(NCORES):
        in_maps.append(
            {
                "wblk16": w16,
                "wblk32": w32,
                "x2": np.ascontiguousarray(
                    x[c * NLOC : (c + 1) * NLOC].T
                ).astype(np.float16),
            }
        )
    return in_maps


def _unshard(res_get):
    U = np.empty((N,), np.float32)
    F = np.empty((N,), np.float32)
    Ft = np.empty((N,), np.float32)
    for c in range(NCORES):
        o = res_get(c)
        U[c * NLOC : (c + 1) * NLOC] = o[0]
        F[c * NLOC : (c + 1) * NLOC] = o[1]
        Ft[c * NLOC : (c + 1) * NLOC] = o[2]
    shp = (B, S, 1)
    return U.reshape(shp), F.reshape(shp), Ft.reshape(shp)


def run(inputs, trace=False):
    nc = _get_nc()
    in_maps = _prep_in_maps(inputs)
    kw = {}
    if trace:
        kw["tmpdir"] = tempfile.mkdtemp(prefix="bassk_prof_")
    res = run_bass_kernel_spmd(
        nc, in_maps, core_ids=list(range(NCORES)), trace=trace, **kw
    )
    return _unshard(lambda c: res.results[c]["out"]), res


def kernel(**inputs):
    outs, _ = run(inputs, trace=False)
    return outs


# ---------------------------------------------------------------------------
# Dev-loop timing: persistent jitted executable (mirrors
# bass2jax.run_bass_via_pjrt's multi-core branch) so repeated executions
# reuse one compiled NEFF and can be timed back-to-back.
# ---------------------------------------------------------------------------
def _make_runner():
    if "runner" in _STATE:
        return _STATE["runner"]
    import jax
    from jax.experimental.shard_map import shard_map
    from jax.sharding import Mesh, PartitionSpec
    from concourse import bass2jax

    bass2jax.install_neuronx_cc_hook()
    nc = _get_nc()

    in_names, out_names, out_avals, zero_outs = [], [], [], []
    for alloc in nc.m.functions[0].allocations:
        if not isinstance(alloc, mybir.MemoryLocationSet):
            continue
        name = alloc.memorylocations[0].name
        if alloc.kind == "ExternalInput":
            if nc.partition_id_tensor is None or name != nc.partition_id_tensor.name:
                in_names.append(name)
        elif alloc.kind == "ExternalOutput":
            out_names.append(name)
            shape = tuple(alloc.tensor_shape)
            dtype = mybir.dt.np(alloc.dtype)
            out_avals.append(jax.core.ShapedArray(shape, dtype))
            zero_outs.append(np.zeros(shape, dtype))
    n_params = len(in_names)
    n_outs = len(out_avals)
    all_names = in_names + out_names
    if nc.partition_id_tensor is not None:
        all_names = all_names + [nc.partition_id_tensor.name]

    def _body(*args):
        operands = list(args)
        if nc.partition_id_tensor is not None:
            operands.append(bass2jax.partition_id_tensor())
        outs = bass2jax._bass_exec_p.bind(
            *operands,
            out_avals=tuple(out_avals),
            in_names=tuple(all_names),
            out_names=tuple(out_names),
            lowering_input_output_aliases=(),
            sim_require_finite=True,
            sim_require_nnan=True,
            nc=nc,
        )
        return tuple(outs)

    devices = jax.devices()[:NCORES]
    mesh = Mesh(np.asarray(devices), ("core",))
    donate = tuple(range(n_params, n_params + n_outs))
    sharded = jax.jit(
        shard_map(
            _body,
            mesh=mesh,
            in_specs=(PartitionSpec("core"),) * (n_params + n_outs),
            out_specs=(PartitionSpec("core"),) * n_outs,
            check_rep=False,
        ),
        donate_argnums=donate,
        keep_unused=True,
    )
    _STATE["runner"] = (sharded, in_names, out_names, out_avals, zero_outs)
    return _STATE["runner"]


def run_timed(inputs, iters=20):
    """Run via a persistent executable; return (outputs, per_iter_ns)."""
    import time as _time

    import jax

    sharded, in_names, out_names, out_avals, zero_outs = _make_runner()
    in_maps = _prep_in_maps(inputs)
    concat_in = [
        np.concatenate([np.asarray(in_maps[c][n]) for c in range(NCORES)], axis=0)
        for n in in_names
    ]
    dev_in = [jax.device_put(a) for a in concat_in]

    def zeros():
        return [
            np.zeros((NCORES * z.shape[0], *z.shape[1:]), z.dtype) for z in zero_outs
        ]

    # warmup (compiles on first call)
    outs = sharded(*dev_in, *zeros())
    jax.block_until_ready(outs)
    out_np = [np.asarray(o) for o in outs]

    zbufs = [zeros() for _ in range(iters)]
    t0 = _time.perf_counter()
    last = None
    for i in range(iters):
        last = sharded(*dev_in, *zbufs[i])
    jax.block_until_ready(last)
    t1 = _time.perf_counter()
    per_iter_ns = (t1 - t0) / iters * 1e9

    per_core = [
        {
            name: out_np[i].reshape(NCORES, *out_avals[i].shape)[c]
            for i, name in enumerate(out_names)
        }
        for c in range(NCORES)
    ]
    return _unshard(lambda c: per_core[c]["out"]), per_iter_ns


# revision 34
# speedup vs baseline: 1.6254x; 1.0918x over previous
"""Trainium2 Bass kernel for the CapacityNN PINN forward pass (v2).

Computes, for N = B*S collocation points x = (s, t):
  U   = MLP([s_norm, t_norm]) * tgt_std + tgt_mean
  F   = U_t  - G(U)             (G = Verhulst logistic growth term)
  F_t = U_tt - G'(U) * U_t
with U_t/U_tt computed exactly by forward-mode 2nd-order jet propagation
through the tanh MLP.

Sharding: pure data parallel over 8 NeuronCores (8192 points/core),
MLP weights + PDE scalars replicated (host-folded).

v2 layout/engine plan (from TimelineSim cost-model analysis):
  - all streams fp16 (DVE 2x tensor_tensor / 4x tensor_scalar modes)
  - [128, 1024]-wide PSUM tiles -> one Act/DVE op per stream per layer
  - elementwise jet algebra balanced across Act / DVE / Pool:
      Act : tanh, st=2*z1^2 (PSUM reads), 2 of 8 ee squares, final copies
      DVE : ee, dm=1-ee, tt=av*st, h2=dm*qt, ad0, 4 of 6 h1=dm*z1
      Pool: qt=z2-tt (PSUM read), 2 of 6 h1
  - all scalar prep + weight folding done on HOST; weights arrive as two
    pre-packed SBUF-image blocks (2 big DMAs instead of ~60 small ones)
  - tail transpose ([3,NLOC] -> [128,3*PPP]) streamed per-chunk via
    SBUF->SBUF DMAs overlapped with compute
"""

import os
import sys
import tempfile

import numpy as np

for _p in ("/opt/trn_rl_repo", "/root/.axon_site/_ro/trn_rl_repo"):
    if os.path.isdir(_p) and _p not in sys.path:
        sys.path.insert(0, _p)

import concourse.bass as bass
import concourse.bacc as bacc
import concourse.tile as tile
from concourse import mybir
from concourse.bass_utils import run_bass_kernel_spmd

AF = mybir.ActivationFunctionType
OP = mybir.AluOpType
F32 = mybir.dt.float32
F16 = mybir.dt.float16

NCORES = 8
B, S, H = 512, 128, 256
N = B * S                  # 65536 points
NLOC = N // NCORES         # 8192 points per core
CH = 1024                  # points per on-chip chunk
NCHUNK = NLOC // CH
PPP = NLOC // 128          # points per partition in the tail layout (64)
CPC = CH // 128            # tail cols per chunk (8)
SQRT2 = float(np.sqrt(2.0))

# ---- packed fp16 const block column map (must match _pack_w16) ----
# 21 [128,128] weight tiles (incl negid), then w0ts (rows 0-1), then w4 (2 cols)
_W16_TILES = []  # (name, col) in order
_c = 0
for _l in (1, 2, 3):
    for _kk in range(2):
        for _mm in range(2):
            _W16_TILES.append((f"wt{_l}_{_kk}{_mm}", _c))
            _c += 128
for _nm in ("wtw", "wtw2"):
    for _kk in range(2):
        for _mm in range(2):
            _W16_TILES.append((f"{_nm}_{_kk}{_mm}", _c))
            _c += 128
_W16_TILES.append(("negid", _c))
_c += 128
_W16_W0TS = _c          # [2, 256] at rows 0-1, cols [_c, _c+256)
_c += 256
_W16_W4 = _c            # [128, 4]: cols = [w4_k0, w4_k1, w4/sqrt2_k0, w4/sqrt2_k1]
_c += 4
W16COLS = _c

# ---- packed fp32 const block column map (must match _pack_w32) ----
# [128, W32COLS]: per-partition scalars and biases, one col each
_W32_NAMES = [
    "beta0_0", "beta0_1",
    "bl1_0", "bl1_1", "bl2_0", "bl2_1", "bl3_0", "bl3_1",
    "C_t", "c1", "nr", "mc3", "sts", "tmb",
]
W32COLS = len(_W32_NAMES)
_W32_IDX = {n: i for i, n in enumerate(_W32_NAMES)}


def _build():
    nc = bacc.Bacc(
        "TRN2",
        target_bir_lowering=False,
        debug=False,
        enable_asserts=False,
        num_devices=NCORES,
    )

    x2 = nc.dram_tensor("x2", [2, NLOC], F16, kind="ExternalInput").ap()
    wblk16 = nc.dram_tensor("wblk16", [128, W16COLS], F16, kind="ExternalInput").ap()
    wblk32 = nc.dram_tensor("wblk32", [128, W32COLS], F32, kind="ExternalInput").ap()
    out = nc.dram_tensor("out", [3, NLOC], F32, kind="ExternalOutput").ap()

    with tile.TileContext(nc) as tc:
        from contextlib import ExitStack

        with ExitStack() as ctx:
            const = ctx.enter_context(tc.tile_pool(name="const", bufs=1))
            sb = ctx.enter_context(tc.tile_pool(name="sb", bufs=1))
            ps = ctx.enter_context(tc.tile_pool(name="ps", bufs=1, space="PSUM"))

            # ---------- const loads ----------
            # split so layer-0's weights (w0ts, at the tail of the block)
            # arrive before the bulk of the hidden-layer tiles
            w16 = const.tile([128, W16COLS], F16, name="w16")
            w32 = const.tile([128, W32COLS], F32, name="w32")
            _SPLIT = _W16_TILES[-1][1]  # negid col: negid+w0ts+w4 in first DMA
            nc.sync.dma_start(out=w16[:, _SPLIT:], in_=wblk16[:, _SPLIT:])
            nc.sync.dma_start(out=w32, in_=wblk32)
            # bulk hidden-layer weights stream in behind the first x2c loads
            _wbulk = [False]

            def load_wbulk():
                if not _wbulk[0]:
                    _wbulk[0] = True
                    nc.sync.dma_start(out=w16[:, :_SPLIT], in_=wblk16[:, :_SPLIT])

            wtile = {}
            for nm, col in _W16_TILES:
                wtile[nm] = w16[:, col : col + 128]
            w0ts = w16[0:2, _W16_W0TS : _W16_W0TS + 256]  # [2, 256]
            w4c = [w16[:, _W16_W4 + kk : _W16_W4 + kk + 1] for kk in range(4)]

            def sc(name):
                i = _W32_IDX[name]
                return w32[:, i : i + 1]

            beta0 = [sc("beta0_0"), sc("beta0_1")]
            bl = {l: [sc(f"bl{l}_0"), sc(f"bl{l}_1")] for l in (1, 2, 3)}
            C_t, c1, nr, mc3, sts, tmb = (
                sc("C_t"), sc("c1"), sc("nr"), sc("mc3"), sc("sts"), sc("tmb"),
            )

            # ---------- main loop: software-pipelined chunk PAIRS ----------
            # PE/Act/DVE/Pool execute their queues in order, so matmuls and
            # elementwise consumers of the two chunks in a pair are emitted
            # interleaved: while chunk A's tanh->ee->dm->h1 chain drains,
            # the PE runs chunk B's matmuls (keeps the PE p-state ramped).
            # oc[p, c*24 + b*3 + s] = stream-s output for point c*CH + b*128 + p
            oc = sb.tile([128, 3 * PPP], F32, name="oc")
            oc2 = sb.tile([128, 3 * PPP], F32, name="oc2")

            PSB = 4   # psum [128,1024] ring (2 banks each -> all 8 banks)
            st8 = {}  # per-chunk live tiles

            def new_stream(c, tag, bufs=4):
                return sb.tile([128, CH], F16, tag=tag, bufs=bufs, name=tag)

            def l0_mm(c):
                x2c = sb.tile([2, CH], F16, tag="x2c", bufs=3)
                nc.sync.dma_start(out=x2c, in_=x2[:, c * CH : (c + 1) * CH])
                pzs = []
                for m in range(2):
                    pz = ps.tile([128, CH], F32, tag="pz", bufs=PSB, name="pz0")
                    for g in range(2):
                        nc.tensor.matmul(
                            pz[:, g * 512 : (g + 1) * 512],
                            w0ts[:, m * 128 : (m + 1) * 128],
                            x2c[:, g * 512 : (g + 1) * 512],
                            start=True,
                            stop=True,
                        )
                    pzs.append(pz)
                st8[c] = {"pz": pzs}

            def l0_cons(c):
                s = st8[c]
                Hv, H1, H2 = [None] * 2, [None] * 2, [None] * 2
                for m in range(2):
                    av = new_stream(c, f"hv{m}")
                    ee = new_stream(c, f"ee{m}", 3)
                    dm = new_stream(c, f"dm{m}", 3)
                    ad = new_stream(c, f"ad{m}", 3)
                    nc.scalar.activation(av, s["pz"][m], AF.Tanh, beta0[m])
                    nc.vector.tensor_tensor(ee, av, av, OP.mult)
                    nc.vector.tensor_scalar(dm, ee, -1.0, 1.0, OP.mult, OP.add)
                    nc.gpsimd.tensor_tensor(ad, av, dm, OP.mult)
                    Hv[m], H1[m], H2[m] = av, dm, ad
                s["Hv"], s["H1"], s["H2"] = Hv, H1, H2

            def prim_mm(c, l):
                s = st8[c]
                s["pz"] = []
                for m in range(2):
                    pz = ps.tile([128, CH], F32, tag="pz", bufs=PSB, name="pzv")
                    for g in range(2):
                        for kk in range(2):
                            nc.tensor.matmul(
                                pz[:, g * 512 : (g + 1) * 512],
                                wtile[f"wt{l}_{kk}{m}"],
                                s["Hv"][kk][:, g * 512 : (g + 1) * 512],
                                start=(kk == 0),
                                stop=(kk == 1),
                            )
                    s["pz"].append(pz)

            def tanh_emit(c, l):
                s = st8[c]
                s["nHv"] = []
                for m in range(2):
                    av = new_stream(c, f"hv{m}")
                    nc.scalar.activation(av, s["pz"][m], AF.Tanh, bl[l][m])
                    s["nHv"].append(av)

            def d1_mm(c, l):
                s = st8[c]
                s["pz1"] = []
                for m in range(2):
                    pz1 = ps.tile([128, CH], F32, tag="pz", bufs=PSB, name="pz1")
                    w1nm = f"wtw_{{}}{m}" if l == 1 else f"wt{l}_{{}}{m}"
                    for g in range(2):
                        for kk in range(2):
                            nc.tensor.matmul(
                                pz1[:, g * 512 : (g + 1) * 512],
                                wtile[w1nm.format(kk)],
                                s["H1"][kk][:, g * 512 : (g + 1) * 512],
                                start=(kk == 0),
                                stop=(kk == 1),
                            )
                    s["pz1"].append(pz1)

            def z1c_emit(c, l):
                # evacuate z1 from PSUM via one fast Act copy; frees the pz1
                # psum slot early (the old st/h1 psum reads held it ~6us).
                # The H1 stream carries a sqrt2 scale (host-folded into wtw;
                # compensated by w4/sqrt2 in the final projection) so that
                # st = z1c^2 = 2*z1_true^2 with no extra scale op.
                s = st8[c]
                s["z1c"] = []
                for m in range(2):
                    z1c = new_stream(c, f"z1c{m}", 3)
                    nc.scalar.copy(z1c, s["pz1"][m])
                    s["z1c"].append(z1c)

            def eedm_emit(c, l):
                s = st8[c]
                s["nDm"] = []
                for m in range(2):
                    ee = new_stream(c, f"ee{m}", 3)
                    dm = new_stream(c, f"dm{m}", 3)
                    if l == 1 or (l == 2 and m == 0):
                        nc.scalar.activation(ee, s["nHv"][m], AF.Square)
                    else:
                        nc.vector.tensor_tensor(ee, s["nHv"][m], s["nHv"][m], OP.mult)
                    nc.vector.tensor_scalar(dm, ee, -1.0, 1.0, OP.mult, OP.add)
                    s["nDm"].append(dm)

            def sth1_emit(c, l):
                # st/tt feed the critical chain st->tt->negid(PE)->h2(DVE):
                # keep them on DVE. h1 only feeds the NEXT layer's d1 matmuls
                # (a full stage of slack) so it rides on the slow Pool engine.
                s = st8[c]
                s["nH1"], s["st"] = [], []
                for m in range(2):
                    st = new_stream(c, f"st{m}", 3)
                    nc.vector.tensor_tensor(st, s["z1c"][m], s["z1c"][m], OP.mult)
                    s["st"].append(st)
                for m in range(2):
                    h1t = new_stream(c, f"h1{m}")
                    nc.gpsimd.tensor_tensor(h1t, s["nDm"][m], s["z1c"][m], OP.mult)
                    s["nH1"].append(h1t)

            def tt_emit(c, l):
                s = st8[c]
                s["Tt"] = []
                for m in range(2):
                    tt = new_stream(c, f"tt{m}", 3)
                    nc.vector.tensor_tensor(tt, s["nHv"][m], s["st"][m], OP.mult)
                    s["Tt"].append(tt)

            def d2_mm(c, l):
                s = st8[c]
                s["pz2"] = []
                for m in range(2):
                    pz2 = ps.tile([128, CH], F32, tag="pz", bufs=PSB, name="pz2")
                    w2nm = f"wtw2_{{}}{m}" if l == 1 else f"wt{l}_{{}}{m}"
                    for g in range(2):
                        for kk in range(2):
                            nc.tensor.matmul(
                                pz2[:, g * 512 : (g + 1) * 512],
                                wtile[w2nm.format(kk)],
                                s["H2"][kk][:, g * 512 : (g + 1) * 512],
                                start=(kk == 0),
                                stop=False,
                            )
                    s["pz2"].append(pz2)

            def negid_mm(c, l):
                # qt = z2 - tt: accumulate -I @ tt into the z2 psum group
                s = st8[c]
                for m in range(2):
                    for g in range(2):
                        nc.tensor.matmul(
                            s["pz2"][m][:, g * 512 : (g + 1) * 512],
                            wtile["negid"],
                            s["Tt"][m][:, g * 512 : (g + 1) * 512],
                            start=False,
                            stop=True,
                        )

            def h2_emit(c, l):
                s = st8[c]
                s["nH2"] = []
                for m in range(2):
                    h2t = new_stream(c, f"h2{m}")
                    nc.vector.tensor_tensor(h2t, s["nDm"][m], s["pz2"][m], OP.mult)
                    s["nH2"].append(h2t)

            def layer_rotate(c):
                s = st8[c]
                s["Hv"], s["H1"], s["H2"] = s["nHv"], s["nH1"], s["nH2"]

            def final_mm(c):
                s = st8[c]
                pblk = ps.tile([128, 3 * CPC], F32, tag="pz", bufs=PSB, name="pblk")
                for b in range(CPC):
                    for s_idx, stream in enumerate((s["Hv"], s["H1"], s["H2"])):
                        for kk in range(2):
                            nc.tensor.matmul(
                                pblk[:, b * 3 + s_idx : b * 3 + s_idx + 1],
                                stream[kk][:, b * 128 : (b + 1) * 128],
                                w4c[(2 if s_idx == 1 else 0) + kk],
                                start=(kk == 0),
                                stop=(kk == 1),
                            )
                s["pblk"] = pblk

            def final_copy(c):
                nc.scalar.copy(
                    oc[:, c * 3 * CPC : (c + 1) * 3 * CPC], st8[c]["pblk"]
                )

            def pair_pde(c0, ng=2):
                # PDE algebra for this group's slice of oc (DVE + Pool for
                # the tensor_tensor ops), then the output DMAs.
                lo = c0 * 3 * CPC
                n = ng * 3 * CPC
                osl = slice(lo, lo + n)
                ocp = oc[:, osl]
                ocq = oc2[:, osl]
                yv = ocp[:, 0:n:3]
                yt = ocp[:, 1:n:3]
                ytt = ocp[:, 2:n:3]
                U = ocq[:, 0:n:3]
                Fo = ocq[:, 1:n:3]
                Ft = ocq[:, 2:n:3]
                k = ng * CPC

                def tl(name):
                    return sb.tile([128, k], F32, tag=name, bufs=2, name=name)

                ut, utt, vv, v2, w1, q1, t1 = (
                    tl("ut"), tl("utt"), tl("vv"), tl("v2"),
                    tl("w1"), tl("q1"), tl("t1"),
                )
                ve = nc.vector
                ve.tensor_scalar(U, yv, sts, tmb, OP.mult, OP.add)
                ve.tensor_scalar(ut, yt, sts, None, OP.mult)
                ve.tensor_scalar(utt, ytt, sts, None, OP.mult)
                ve.tensor_scalar(vv, U, C_t, None, OP.subtract)
                nc.gpsimd.tensor_tensor(v2, vv, vv, OP.mult)
                ve.scalar_tensor_tensor(w1, v2, c1, vv, OP.mult, OP.add)
                ve.scalar_tensor_tensor(Fo, w1, nr, ut, OP.mult, OP.add)
                nc.gpsimd.tensor_tensor(q1, vv, ut, OP.mult)
                ve.scalar_tensor_tensor(t1, ut, nr, utt, OP.mult, OP.add)
                ve.scalar_tensor_tensor(Ft, q1, mc3, t1, OP.mult, OP.add)
                # out[s, c*CH + b*128 + p] = oc2[p, c*3*CPC + b*3 + s]
                for s_idx in range(3):
                    nc.sync.dma_start(
                        out=bass.AP(
                            out.tensor,
                            s_idx * NLOC + c0 * CH,
                            [[1, 128], [CH, ng], [128, CPC]],
                        ),
                        in_=bass.AP(
                            oc2.tensor,
                            oc2.offset + lo + s_idx,
                            [list(oc2.ap[0]), [3 * CPC, ng], [3, CPC]],
                        ),
                    )

            groups = [(0, 1, 2), (3, 4, 5), (6, 7)]
            prev = None
            for grp in groups:
                for c in grp:
                    l0_mm(c)
                load_wbulk()
                for c in grp:
                    l0_cons(c)
                for l in (1, 2, 3):
                    for c in grp:
                        prim_mm(c, l)
                    for c in grp:
                        tanh_emit(c, l)
                    for c in grp:
                        d1_mm(c, l)
                    for c in grp:
                        z1c_emit(c, l)
                    for c in grp:
                        eedm_emit(c, l)
                    for c in grp:
                        sth1_emit(c, l)
                        tt_emit(c, l)
                    last = l == 3
                    for c in grp:
                        d2_mm(c, l)
                        negid_mm(c, l)
                        h2_emit(c, l)
                        layer_rotate(c)
                        if last:
                            final_mm(c)
                    if last:
                        for c in grp:
                            final_copy(c)
                if prev is not None:
                    pair_pde(prev[0], len(prev))
                prev = grp
            pair_pde(prev[0], len(prev))

    nc.compile()
    return nc


_STATE = {}


def _get_nc():
    if "nc" not in _STATE:
        _STATE["nc"] = _build()
    return _STATE["nc"]


def _pack_consts(inputs):
    f = np.float32

    def arr(k):
        return np.ascontiguousarray(np.asarray(inputs[k], f))

    W0, b0 = arr("W0"), arr("b0")
    Ws = {1: arr("W1"), 2: arr("W2"), 3: arr("W3")}
    bs = {1: arr("b1"), 2: arr("b2"), 3: arr("b3")}
    W4, b4 = arr("W4").reshape(1, H), arr("b4").reshape(1)
    in_mean, in_std = arr("in_mean"), arr("in_std")
    tgt_mean, tgt_std = arr("tgt_mean"), arr("tgt_std")
    lgr = float(arr("log_growth_rate").reshape(-1)[0])
    lcc = float(arr("log_carrying_capacity").reshape(-1)[0])
    lil = float(arr("log_initial_loss").reshape(-1)[0])

    # fp16 block
    w16 = np.zeros((128, W16COLS), np.float16)
    for nm, col in _W16_TILES:
        if nm == "negid":
            tilev = -np.eye(128, dtype=np.float32)
        elif nm.startswith("wtw"):
            base, km = nm.rsplit("_", 1)
            kk, mm = int(km[0]), int(km[1])
            if base == "wtw":
                # sqrt2-scaled H1 stream: st = (sqrt2*z1)^2 = 2*z1^2 for free
                Wf = (Ws[1] * (SQRT2 * W0[:, 1])[None, :]).T
            else:
                Wf = (Ws[1] * (-2.0 * W0[:, 1] ** 2)[None, :]).T
            tilev = Wf[kk * 128 : (kk + 1) * 128, mm * 128 : (mm + 1) * 128]
        else:
            l, km = nm[2:].split("_")
            l, kk, mm = int(l), int(km[0]), int(km[1])
            Wt = Ws[l].T  # [in, out]
            tilev = Wt[kk * 128 : (kk + 1) * 128, mm * 128 : (mm + 1) * 128]
        w16[:, col : col + 128] = tilev.astype(np.float16)
    # w0ts: W0.T rows scaled by 1/(std+eps)
    w0ts = (W0.T / (in_std[:, None] + 1e-8)).astype(np.float16)  # [2, H]
    w16[0:2, _W16_W0TS : _W16_W0TS + 256] = w0ts
    # w4 halves; cols 2-3 carry 1/sqrt2 to undo the H1 stream's sqrt2 scale
    for kk in range(2):
        w4h = W4[0, kk * 128 : (kk + 1) * 128]
        w16[:, _W16_W4 + kk] = w4h.astype(np.float16)
        w16[:, _W16_W4 + 2 + kk] = (w4h / SQRT2).astype(np.float16)

    # fp32 block
    w32 = np.zeros((128, W32COLS), np.float32)

    def put(name, vec):
        w32[:, _W32_IDX[name]] = vec

    m0i = in_mean[0] / (in_std[0] + 1e-8)
    m1i = in_mean[1] / (in_std[1] + 1e-8)
    u = W0[:, 0] * m0i + W0[:, 1] * m1i
    beta0 = b0 - u
    put("beta0_0", beta0[0:128])
    put("beta0_1", beta0[128:256])
    for l in (1, 2, 3):
        put(f"bl{l}_0", bs[l][0:128])
        put(f"bl{l}_1", bs[l][128:256])
    r = np.exp(-lgr)
    K = 0.2 + 0.8 / (1.0 + np.exp(-lcc))
    C = 0.1 / (1.0 + np.exp(-lil))
    put("C_t", C)
    put("c1", -1.0 / (K - C))
    put("nr", -r)
    put("mc3", 2.0 * r / (K - C))
    put("sts", tgt_std[0])
    put("tmb", b4[0] * tgt_std[0] + tgt_mean[0])
    return w16, w32


def _prep_in_maps(inputs):
    w16, w32 = _pack_consts(inputs)
    x = np.asarray(inputs["inputs"], np.float32).reshape(N, 2)
    in_maps = []
    for c in range(NCORES):
        in_maps.append(
            {
                "wblk16": w16,
                "wblk32": w32,
                "x2": np.ascontiguousarray(
                    x[c * NLOC : (c + 1) * NLOC].T
                ).astype(np.float16),
            }
        )
    return in_maps


def _unshard(res_get):
    U = np.empty((N,), np.float32)
    F = np.empty((N,), np.float32)
    Ft = np.empty((N,), np.float32)
    for c in range(NCORES):
        o = res_get(c)
        U[c * NLOC : (c + 1) * NLOC] = o[0]
        F[c * NLOC : (c + 1) * NLOC] = o[1]
        Ft[c * NLOC : (c + 1) * NLOC] = o[2]
    shp = (B, S, 1)
    return U.reshape(shp), F.reshape(shp), Ft.reshape(shp)


def run(inputs, trace=False):
    nc = _get_nc()
    in_maps = _prep_in_maps(inputs)
    kw = {}
    if trace:
        kw["tmpdir"] = tempfile.mkdtemp(prefix="bassk_prof_")
    res = run_bass_kernel_spmd(
        nc, in_maps, core_ids=list(range(NCORES)), trace=trace, **kw
    )
    return _unshard(lambda c: res.results[c]["out"]), res


def kernel(**inputs):
    outs, _ = run(inputs, trace=False)
    return outs


# ---------------------------------------------------------------------------
# Dev-loop timing: persistent jitted executable (mirrors
# bass2jax.run_bass_via_pjrt's multi-core branch) so repeated executions
# reuse one compiled NEFF and can be timed back-to-back.
# ---------------------------------------------------------------------------
def _make_runner():
    if "runner" in _STATE:
        return _STATE["runner"]
    import jax
    from jax.experimental.shard_map import shard_map
    from jax.sharding import Mesh, PartitionSpec
    from concourse import bass2jax

    bass2jax.install_neuronx_cc_hook()
    nc = _get_nc()

    in_names, out_names, out_avals, zero_outs = [], [], [], []
    for alloc in nc.m.functions[0].allocations:
        if not isinstance(alloc, mybir.MemoryLocationSet):
            continue
        name = alloc.memorylocations[0].name
        if alloc.kind == "ExternalInput":
            if nc.partition_id_tensor is None or name != nc.partition_id_tensor.name:
                in_names.append(name)
        elif alloc.kind == "ExternalOutput":
            out_names.append(name)
            shape = tuple(alloc.tensor_shape)
            dtype = mybir.dt.np(alloc.dtype)
            out_avals.append(jax.core.ShapedArray(shape, dtype))
            zero_outs.append(np.zeros(shape, dtype))
    n_params = len(in_names)
    n_outs = len(out_avals)
    all_names = in_names + out_names
    if nc.partition_id_tensor is not None:
        all_names = all_names + [nc.partition_id_tensor.name]

    def _body(*args):
        operands = list(args)
        if nc.partition_id_tensor is not None:
            operands.append(bass2jax.partition_id_tensor())
        outs = bass2jax._bass_exec_p.bind(
            *operands,
            out_avals=tuple(out_avals),
            in_names=tuple(all_names),
            out_names=tuple(out_names),
            lowering_input_output_aliases=(),
            sim_require_finite=True,
            sim_require_nnan=True,
            nc=nc,
        )
        return tuple(outs)

    devices = jax.devices()[:NCORES]
    mesh = Mesh(np.asarray(devices), ("core",))
    donate = tuple(range(n_params, n_params + n_outs))
    sharded = jax.jit(
        shard_map(
            _body,
            mesh=mesh,
            in_specs=(PartitionSpec("core"),) * (n_params + n_outs),
            out_specs=(PartitionSpec("core"),) * n_outs,
            check_rep=False,
        ),
        donate_argnums=donate,
        keep_unused=True,
    )
    _STATE["runner"] = (sharded, in_names, out_names, out_avals, zero_outs)
    return _STATE["runner"]


def run_timed(inputs, iters=20):
    """Run via a persistent executable; return (outputs, per_iter_ns)."""
    import time as _time

    import jax

    sharded, in_names, out_names, out_avals, zero_outs = _make_runner()
    in_maps = _prep_in_maps(inputs)
    concat_in = [
        np.concatenate([np.asarray(in_maps[c][n]) for c in range(NCORES)], axis=0)
        for n in in_names
    ]
    dev_in = [jax.device_put(a) for a in concat_in]

    def zeros():
        return [
            np.zeros((NCORES * z.shape[0], *z.shape[1:]), z.dtype) for z in zero_outs
        ]

    # warmup (compiles on first call)
    outs = sharded(*dev_in, *zeros())
    jax.block_until_ready(outs)
    out_np = [np.asarray(o) for o in outs]

    zbufs = [zeros() for _ in range(iters)]
    t0 = _time.perf_counter()
    last = None
    for i in range(iters):
        last = sharded(*dev_in, *zbufs[i])
    jax.block_until_ready(last)
    t1 = _time.perf_counter()
    per_iter_ns = (t1 - t0) / iters * 1e9

    per_core = [
        {
            name: out_np[i].reshape(NCORES, *out_avals[i].shape)[c]
            for i, name in enumerate(out_names)
        }
        for c in range(NCORES)
    ]
    return _unshard(lambda c: per_core[c]["out"]), per_iter_ns
